# revision 7
# baseline (speedup 1.0000x reference)
"""Mixtral MoE layer (top-2 of 8 experts) on 8 Trainium2 NeuronCores.

Strategy: expert parallelism. Core e owns expert e's weights (w1/w3/w2[e]).
Each core:
  1. Router (exact fp32): logits = h @ gate_w, top-2 via max8, combine weight
     for own expert via sigmoid(l_e - l_other); builds a compaction rank for
     the tokens routed to this expert (matmul-based prefix sums).
  2. Compaction: payload rows [h | combine | token_id] are indirect-DMA
     scattered into a dense per-expert buffer h_c (capacity TCAP).
  3. FFN over compact tokens (fp32r stage A, bf16 stage B), scaled by the
     combine weight, indirect-scattered to the token's row of a [T,H] buffer.
  4. ReduceScatter(add) across the 8 cores; host concatenates the shards.
"""
import sys

sys.path.insert(0, "/opt/trn_rl_repo")

import numpy as np

import concourse.bass as bass
import concourse.mybir as mybir
from concourse import bacc
from concourse.tile import TileContext
from concourse.masks import make_identity
from concourse.bass_utils import run_bass_kernel_spmd

F32 = mybir.dt.float32
F32R = mybir.dt.float32r
BF16 = mybir.dt.bfloat16
I32 = mybir.dt.int32
AF = mybir.ActivationFunctionType
P = 128


def build_kernel(T=16384, H=1024, FF=3584, E=8, TCAP=4608, CH=512, n_cores=8):
    NT = T // P      # token tiles
    KH = H // P      # contraction tiles over H
    KF = FF // P     # f tiles (stage A output tiles / stage B contraction)
    NCH = TCAP // CH
    CT = CH // P     # token tiles per FFN chunk
    WPAY = H + 8     # payload row: h | combine | token_id | pad
    TRASH = float(T)  # scatter row for capacity-pad slots
    BIG = 1.0e9
    NHALF = max(1, H // 512)  # stage B free-dim chunks
    HW2 = H // NHALF

    nc = bacc.Bacc(num_devices=n_cores)

    h_ext = nc.dram_tensor("h", [T, H], F32, kind="ExternalInput")
    gw_ext = nc.dram_tensor("gate_w", [H, E], F32, kind="ExternalInput")
    w1_ext = nc.dram_tensor("w1l", [H, FF], F32R, kind="ExternalInput")
    w3_ext = nc.dram_tensor("w3l", [H, FF], F32R, kind="ExternalInput")
    w2_ext = nc.dram_tensor("w2l", [FF, H], F32, kind="ExternalInput")
    oh_ext = nc.dram_tensor("onehot", [P, E], F32, kind="ExternalInput")
    out_ext = nc.dram_tensor("out_shard", [T // n_cores, H], F32, kind="ExternalOutput")

    h_c = nc.dram_tensor("h_c", [TCAP, WPAY], F32)
    scat = nc.dram_tensor("scat", [T + P, H], F32)
    rs_out = nc.dram_tensor("rs_out", [T // n_cores, H], F32)

    tok_ids = np.arange(T, dtype=np.float32).reshape(NT, P).T.copy()  # [P, NT]
    tok_const = nc.inline_tensor(tok_ids, name="tok_ids")
    ustrict_np = np.triu(np.ones((P, P), dtype=np.float32), 1)  # [k, m] = 1 iff k < m
    ustrict_const = nc.inline_tensor(ustrict_np, name="ustrict")

    with TileContext(nc) as tc:
        with tc.tile_pool(name="const", bufs=1) as cpool:
            ident = cpool.tile([P, P], F32)
            make_identity(nc, ident[:])
            ustrict = cpool.tile([P, P], F32)
            nc.sync.dma_start(out=ustrict[:], in_=ustrict_const[:])
            tok_slab = cpool.tile([P, NT], F32)
            nc.sync.dma_start(out=tok_slab[:], in_=tok_const[:])
            ones_col = cpool.tile([P, 1], F32)
            nc.vector.memset(ones_col[:], 1.0)
            ones_row = cpool.tile([1, P], F32)
            nc.vector.memset(ones_row[:], 1.0)
            gw_sb = cpool.tile([P, KH, E], F32)
            nc.sync.dma_start(out=gw_sb[:], in_=gw_ext[:].rearrange("(k p) e -> p k e", p=P))
            oh_sb = cpool.tile([P, E], F32)
            nc.sync.dma_start(out=oh_sb[:], in_=oh_ext[:])
            zrow = cpool.tile([P, WPAY], F32)
            nc.vector.memset(zrow[:], 0.0)
            nc.vector.memset(zrow[:, H + 1:H + 2], TRASH)

            # -------- zero-fill h_c and scat --------
            for r in range(TCAP // P):
                nc.sync.dma_start(out=h_c[r * P:(r + 1) * P, :], in_=zrow[:])
            for r in range((T + P) // P):
                nc.sync.dma_start(out=scat[r * P:(r + 1) * P, :], in_=zrow[:, 0:H])

            # -------- router + compaction slabs --------
            with tc.tile_pool(name="rslab", bufs=1) as spool:
                lg_slab = spool.tile([P, NT, E], F32)
                mx_slab = spool.tile([P, NT, 8], F32)
                mask_slab = spool.tile([P, NT], F32)
                comb_slab = spool.tile([P, NT], F32)
                rank_i = spool.tile([P, NT], I32)

                with tc.tile_pool(name="rtile", bufs=3) as rpool, \
                     tc.tile_pool(name="rpsum", bufs=2, space="PSUM") as rpsum, \
                     tc.tile_pool(name="rcpsum", bufs=1, space="PSUM") as rcpsum:
                    for i in range(NT):
                        ht = rpool.tile([P, H], F32, tag="ht")
                        nc.sync.dma_start(out=ht[:], in_=h_ext[i * P:(i + 1) * P, :])
                        trp = rpsum.tile([P, KH, P], F32, tag="trp")
                        for k in range(KH):
                            nc.tensor.transpose(out=trp[:, k], in_=ht[:, k * P:(k + 1) * P],
                                                identity=ident[:])
                        hTt = rpool.tile([P, KH, P], F32, tag="hT")
                        nc.vector.tensor_copy(out=hTt[:], in_=trp[:])
                        lg = rpsum.tile([P, E], F32, tag="lg")
                        for k in range(KH):
                            nc.tensor.matmul(lg[:], lhsT=hTt[:, k], rhs=gw_sb[:, k],
                                             start=(k == 0), stop=(k == KH - 1))
                        nc.scalar.copy(out=lg_slab[:, i], in_=lg[:])
                        nc.vector.max(out=mx_slab[:, i], in_=lg_slab[:, i])

                    # batched combine/mask over the full slabs
                    tmp_le = spool.tile([P, NT, E], F32)
                    nc.vector.tensor_mul(out=tmp_le[:], in0=lg_slab[:],
                                          in1=oh_sb[:, None, :].to_broadcast([P, NT, E]))
                    le = spool.tile([P, NT], F32)
                    nc.vector.tensor_reduce(out=le[:], in_=tmp_le[:],
                                            axis=mybir.AxisListType.X,
                                            op=mybir.AluOpType.add)
                    m1 = mx_slab[:, :, 0]
                    m2 = mx_slab[:, :, 1]
                    msum = spool.tile([P, NT], F32)
                    nc.vector.tensor_add(out=msum[:], in0=m1, in1=m2)
                    sgin = spool.tile([P, NT], F32)
                    nc.vector.tensor_scalar_mul(sgin[:], le[:], 2.0)
                    nc.vector.tensor_sub(out=sgin[:], in0=sgin[:], in1=msum[:])
                    sig = spool.tile([P, NT], F32)
                    nc.scalar.activation(sig[:], sgin[:], AF.Sigmoid)
                    eq1 = spool.tile([P, NT], F32)
                    eq2 = spool.tile([P, NT], F32)
                    nc.vector.tensor_tensor(out=eq1[:], in0=le[:], in1=m1,
                                            op=mybir.AluOpType.is_equal)
                    nc.vector.tensor_tensor(out=eq2[:], in0=le[:], in1=m2,
                                            op=mybir.AluOpType.is_equal)
                    nc.vector.tensor_add(out=mask_slab[:], in0=eq1[:], in1=eq2[:])
                    nc.vector.tensor_mul(out=comb_slab[:], in0=mask_slab[:], in1=sig[:])

                    # ---- compaction ranks (exclusive prefix of mask in token order) ----
                    csum_ps = rcpsum.tile([1, NT], F32, tag="c1")
                    nc.tensor.matmul(csum_ps[:], lhsT=ones_col[:], rhs=mask_slab[:],
                                     start=True, stop=True)
                    cs = spool.tile([1, NT], F32)
                    nc.vector.tensor_copy(out=cs[:], in_=csum_ps[:])
                    zer = spool.tile([1, NT], F32)
                    nc.vector.memset(zer[:], 0.0)
                    incl = spool.tile([1, NT], F32)
                    nc.vector.tensor_tensor_scan(out=incl[:], data0=cs[:], data1=zer[:],
                                                 initial=0.0,
                                                 op0=mybir.AluOpType.add,
                                                 op1=mybir.AluOpType.add)
                    cpref_row = spool.tile([1, NT], F32)
                    nc.vector.tensor_sub(out=cpref_row[:], in0=incl[:], in1=cs[:])

                    rank_ps = rcpsum.tile([P, NT], F32, tag="rk")
                    nc.tensor.matmul(rank_ps[:], lhsT=ustrict[:], rhs=mask_slab[:],
                                     start=True, stop=False)
                    nc.tensor.matmul(rank_ps[:], lhsT=ones_row[:], rhs=cpref_row[:],
                                     start=False, stop=True)
                    pad_off = spool.tile([P, NT], F32)
                    nc.vector.tensor_scalar(out=pad_off[:], in0=mask_slab[:],
                                            scalar1=-BIG, scalar2=BIG,
                                            op0=mybir.AluOpType.mult,
                                            op1=mybir.AluOpType.add)
                    rank_f = spool.tile([P, NT], F32)
                    nc.vector.tensor_add(out=rank_f[:], in0=rank_ps[:], in1=pad_off[:])
                    nc.vector.tensor_copy(out=rank_i[:], in_=rank_f[:])

                # -------- payload scatter pass --------
                with tc.tile_pool(name="ppool", bufs=4) as ppool:
                    for i in range(NT):
                        pay = ppool.tile([P, WPAY], F32, tag="pay")
                        nc.sync.dma_start(out=pay[:, 0:H], in_=h_ext[i * P:(i + 1) * P, :])
                        nc.vector.tensor_copy(out=pay[:, H:H + 1], in_=comb_slab[:, i:i + 1])
                        nc.vector.tensor_copy(out=pay[:, H + 1:H + 2], in_=tok_slab[:, i:i + 1])
                        nc.gpsimd.indirect_dma_start(
                            out=h_c[:],
                            out_offset=bass.IndirectOffsetOnAxis(ap=rank_i[:, i:i + 1], axis=0),
                            in_=pay[:], in_offset=None,
                            bounds_check=TCAP - 1, oob_is_err=False)

            # -------- FFN over compact tokens --------
            with tc.tile_pool(name="fpool", bufs=2) as fpool, \
                 tc.tile_pool(name="hcpool", bufs=CT + 2) as hcpool, \
                 tc.tile_pool(name="gpool", bufs=KF) as gpool, \
                 tc.tile_pool(name="w2pool", bufs=KF) as w2pool, \
                 tc.tile_pool(name="opool", bufs=3) as opool, \
                 tc.tile_pool(name="ftrpsum", bufs=1, space="PSUM") as ftrpsum, \
                 tc.tile_pool(name="fpsum", bufs=2, space="PSUM") as fpsum, \
                 tc.tile_pool(name="opsum", bufs=1, space="PSUM") as opsum:

                # w2 resident in bf16
                w2b = []
                for f in range(KF):
                    w2s = fpool.tile([P, H], F32, tag="w2stage")
                    nc.sync.dma_start(out=w2s[:], in_=w2_ext[f * P:(f + 1) * P, :])
                    w2t = w2pool.tile([P, H], BF16, tag="w2b")
                    nc.vector.tensor_copy(out=w2t[:], in_=w2s[:])
                    w2b.append(w2t)

                for c in range(NCH):
                    hcts = []
                    idxs = []
                    for t in range(CT):
                        hct = hcpool.tile([P, WPAY], F32, tag="hc")
                        r0 = c * CH + t * P
                        nc.sync.dma_start(out=hct[:], in_=h_c[r0:r0 + P, :])
                        idx = hcpool.tile([P, 1], I32, tag="idx")
                        nc.vector.tensor_copy(out=idx[:], in_=hct[:, H + 1:H + 2])
                        hcts.append(hct)
                        idxs.append(idx)
                    hTr = fpool.tile([P, KH, CH], F32R, tag="hTr")
                    for t in range(CT):
                        trp = ftrpsum.tile([P, KH, P], F32, tag="ftr")
                        for k in range(KH):
                            nc.tensor.transpose(out=trp[:, k], in_=hcts[t][:, k * P:(k + 1) * P],
                                                identity=ident[:])
                        nc.vector.tensor_copy(out=hTr[:, :, t * P:(t + 1) * P], in_=trp[:])

                    # stage A: G^T tiles [f, tokens]
                    gts = []
                    for f in range(KF):
                        w1s = fpool.tile([P, KH, P], F32R, tag="w1s")
                        nc.sync.dma_start(
                            out=w1s[:],
                            in_=w1_ext[:, f * P:(f + 1) * P].rearrange("(k p) m -> p k m", p=P))
                        w3s = fpool.tile([P, KH, P], F32R, tag="w3s")
                        nc.sync.dma_start(
                            out=w3s[:],
                            in_=w3_ext[:, f * P:(f + 1) * P].rearrange("(k p) m -> p k m", p=P))
                        x1 = fpsum.tile([P, CH], F32, tag="x1")
                        x3 = fpsum.tile([P, CH], F32, tag="x3")
                        for k in range(KH):
                            nc.tensor.matmul(x1[:], lhsT=w1s[:, k], rhs=hTr[:, k],
                                             start=(k == 0), stop=(k == KH - 1))
                        for k in range(KH):
                            nc.tensor.matmul(x3[:], lhsT=w3s[:, k], rhs=hTr[:, k],
                                             start=(k == 0), stop=(k == KH - 1))
                        gate = fpool.tile([P, CH], F32, tag="gate")
                        nc.scalar.activation(gate[:], x1[:], AF.Silu)
                        gt = gpool.tile([P, CH], BF16, tag="G")
                        nc.vector.tensor_mul(out=gt[:], in0=gate[:], in1=x3[:])
                        gts.append(gt)

                    # stage B: out rows, scaled by combine, scattered to scat
                    for t in range(CT):
                        o = opsum.tile([P, H], F32, tag="o")
                        for f in range(KF):
                            for hh in range(NHALF):
                                nc.tensor.matmul(
                                    o[:, hh * HW2:(hh + 1) * HW2],
                                    lhsT=gts[f][:, t * P:(t + 1) * P],
                                    rhs=w2b[f][:, hh * HW2:(hh + 1) * HW2],
                                    start=(f == 0), stop=(f == KF - 1))
                        osb = opool.tile([P, H], F32, tag="osb")
                        nc.vector.tensor_scalar_mul(osb[:], o[:], hcts[t][:, H:H + 1])
                        nc.gpsimd.indirect_dma_start(
                            out=scat[:],
                            out_offset=bass.IndirectOffsetOnAxis(ap=idxs[t][:, 0:1], axis=0),
                            in_=osb[:], in_offset=None,
                            bounds_check=T + P - 1, oob_is_err=False)

            # -------- collective + output --------
            with tc.tile_pool(name="oc", bufs=2) as ocpool:
                nc.gpsimd.collective_compute(
                    "ReduceScatter", mybir.AluOpType.add,
                    replica_groups=[list(range(n_cores))],
                    ins=[scat[0:T, :]], outs=[rs_out[:]])
                shard = T // n_cores
                for r in range(shard // P):
                    oct_ = ocpool.tile([P, H], F32, tag="oct")
                    nc.sync.dma_start(out=oct_[:], in_=rs_out[r * P:(r + 1) * P, :])
                    nc.sync.dma_start(out=out_ext[r * P:(r + 1) * P, :], in_=oct_[:])

    nc.finalize()
    return nc


def kernel(hidden_states, gate_w, w1, w3, w2):
    T, H = hidden_states.shape
    E, _, FF = w1.shape
    n_cores = 8
    nc = build_kernel(T=T, H=H, FF=FF, E=E, n_cores=n_cores)
    onehots = np.eye(E, dtype=np.float32)
    in_maps = []
    for e in range(n_cores):
        in_maps.append({
            "h": np.ascontiguousarray(hidden_states, dtype=np.float32),
            "gate_w": np.ascontiguousarray(gate_w, dtype=np.float32),
            "w1l": np.ascontiguousarray(w1[e], dtype=np.float32),
            "w3l": np.ascontiguousarray(w3[e], dtype=np.float32),
            "w2l": np.ascontiguousarray(w2[e], dtype=np.float32),
            "onehot": np.tile(onehots[e], (128, 1)),
        })
    res = run_bass_kernel_spmd(nc, in_maps, list(range(n_cores))).results
    return np.concatenate([res[i]["out_shard"] for i in range(n_cores)], axis=0)


if __name__ == "__main__":
    nc = build_kernel()
    print("built", len(nc.inst_map), "instructions")


# revision 8
# speedup vs baseline: 1.1149x; 1.1149x over previous
"""Mixtral MoE layer (top-2 of 8 experts) on 8 Trainium2 NeuronCores.

Strategy: expert parallelism. Core e owns expert e's weights (w1/w3/w2[e]).
Each core:
  1. Router (exact fp32): logits = h @ gate_w, top-2 via max8, combine weight
     for own expert via sigmoid(l_e - l_other); builds a compaction rank for
     the tokens routed to this expert (matmul-based prefix sums).
  2. Compaction: payload rows [h | combine | token_id] are indirect-DMA
     scattered into a dense per-expert buffer h_c (capacity TCAP).
  3. FFN over compact tokens (fp32r stage A, bf16 stage B), scaled by the
     combine weight, indirect-scattered to the token's row of a [T,H] buffer.
  4. ReduceScatter(add) across the 8 cores; host concatenates the shards.
"""
import sys

sys.path.insert(0, "/opt/trn_rl_repo")

import numpy as np

import concourse.bass as bass
import concourse.mybir as mybir
from concourse import bacc
from concourse.tile import TileContext
from concourse.masks import make_identity
from concourse.bass_utils import run_bass_kernel_spmd

F32 = mybir.dt.float32
F32R = mybir.dt.float32r
BF16 = mybir.dt.bfloat16
I32 = mybir.dt.int32
AF = mybir.ActivationFunctionType
P = 128


def build_kernel(T=16384, H=1024, FF=3584, E=8, TCAP=4608, CH=512, n_cores=8):
    NT = T // P      # token tiles
    KH = H // P      # contraction tiles over H
    KF = FF // P     # f tiles (stage A output tiles / stage B contraction)
    NCH = TCAP // CH
    CT = CH // P     # token tiles per FFN chunk
    WPAY = H + 8     # payload row: h | combine | token_id | pad
    TRASH = float(T)  # scatter row for capacity-pad slots
    BIG = 1.0e9
    NHALF = max(1, H // 512)  # stage B free-dim chunks
    HW2 = H // NHALF

    nc = bacc.Bacc(num_devices=n_cores)

    h_ext = nc.dram_tensor("h", [T, H], F32, kind="ExternalInput")
    gw_ext = nc.dram_tensor("gate_w", [H, E], F32, kind="ExternalInput")
    w1_ext = nc.dram_tensor("w1l", [H, FF], F32R, kind="ExternalInput")
    w3_ext = nc.dram_tensor("w3l", [H, FF], F32R, kind="ExternalInput")
    w2_ext = nc.dram_tensor("w2l", [FF, H], F32, kind="ExternalInput")
    oh_ext = nc.dram_tensor("onehot", [P, E], F32, kind="ExternalInput")
    out_ext = nc.dram_tensor("out_shard", [T // n_cores, H], F32, kind="ExternalOutput")

    h_c = nc.dram_tensor("h_c", [TCAP, WPAY], F32)
    scat = nc.dram_tensor("scat", [T + P, H], F32)
    rs_out = nc.dram_tensor("rs_out", [T // n_cores, H], F32)

    tok_ids = np.arange(T, dtype=np.float32).reshape(NT, P).T.copy()  # [P, NT]
    tok_const = nc.inline_tensor(tok_ids, name="tok_ids")
    ustrict_np = np.triu(np.ones((P, P), dtype=np.float32), 1)  # [k, m] = 1 iff k < m
    ustrict_const = nc.inline_tensor(ustrict_np, name="ustrict")

    with TileContext(nc) as tc:
        with tc.tile_pool(name="const", bufs=1) as cpool:
            ident = cpool.tile([P, P], F32)
            make_identity(nc, ident[:])
            ustrict = cpool.tile([P, P], F32)
            nc.sync.dma_start(out=ustrict[:], in_=ustrict_const[:])
            tok_slab = cpool.tile([P, NT], F32)
            nc.sync.dma_start(out=tok_slab[:], in_=tok_const[:])
            ones_col = cpool.tile([P, 1], F32)
            nc.vector.memset(ones_col[:], 1.0)
            ones_row = cpool.tile([1, P], F32)
            nc.vector.memset(ones_row[:], 1.0)
            gw_sb = cpool.tile([P, KH, E], F32)
            nc.sync.dma_start(out=gw_sb[:], in_=gw_ext[:].rearrange("(k p) e -> p k e", p=P))
            oh_sb = cpool.tile([P, E], F32)
            nc.sync.dma_start(out=oh_sb[:], in_=oh_ext[:])
            zrow = cpool.tile([P, WPAY], F32)
            nc.vector.memset(zrow[:], 0.0)
            nc.vector.memset(zrow[:, H + 1:H + 2], TRASH)

            # -------- router + compaction slabs --------
            with tc.tile_pool(name="rslab", bufs=1) as spool:
                lg_slab = spool.tile([P, NT, E], F32)
                mx_slab = spool.tile([P, NT, 8], F32)
                mask_slab = spool.tile([P, NT], F32)
                comb_slab = spool.tile([P, NT], F32)
                rank_i = spool.tile([P, NT], I32)

                with tc.tile_pool(name="rtile", bufs=3) as rpool, \
                     tc.tile_pool(name="rpsum", bufs=2, space="PSUM") as rpsum, \
                     tc.tile_pool(name="rcpsum", bufs=1, space="PSUM") as rcpsum:
                    for i in range(NT):
                        ht = rpool.tile([P, H], F32, tag="ht")
                        nc.sync.dma_start(out=ht[:], in_=h_ext[i * P:(i + 1) * P, :])
                        trp = rpsum.tile([P, KH, P], F32, tag="trp")
                        for k in range(KH):
                            nc.tensor.transpose(out=trp[:, k], in_=ht[:, k * P:(k + 1) * P],
                                                identity=ident[:])
                        hTt = rpool.tile([P, KH, P], F32, tag="hT")
                        if i % 2 == 0:
                            nc.vector.tensor_copy(out=hTt[:], in_=trp[:])
                        else:
                            nc.scalar.copy(out=hTt[:], in_=trp[:])
                        lg = rpsum.tile([P, E], F32, tag="lg")
                        for k in range(KH):
                            nc.tensor.matmul(lg[:], lhsT=hTt[:, k], rhs=gw_sb[:, k],
                                             start=(k == 0), stop=(k == KH - 1))
                        nc.scalar.copy(out=lg_slab[:, i], in_=lg[:])
                        nc.vector.max(out=mx_slab[:, i], in_=lg_slab[:, i])

                    # batched combine/mask over the full slabs
                    tmp_le = spool.tile([P, NT, E], F32)
                    nc.vector.tensor_mul(out=tmp_le[:], in0=lg_slab[:],
                                          in1=oh_sb[:, None, :].to_broadcast([P, NT, E]))
                    le = spool.tile([P, NT], F32)
                    nc.vector.tensor_reduce(out=le[:], in_=tmp_le[:],
                                            axis=mybir.AxisListType.X,
                                            op=mybir.AluOpType.add)
                    m1 = mx_slab[:, :, 0]
                    m2 = mx_slab[:, :, 1]
                    msum = spool.tile([P, NT], F32)
                    nc.vector.tensor_add(out=msum[:], in0=m1, in1=m2)
                    sgin = spool.tile([P, NT], F32)
                    nc.vector.tensor_scalar_mul(sgin[:], le[:], 2.0)
                    nc.vector.tensor_sub(out=sgin[:], in0=sgin[:], in1=msum[:])
                    sig = spool.tile([P, NT], F32)
                    nc.scalar.activation(sig[:], sgin[:], AF.Sigmoid)
                    eq1 = spool.tile([P, NT], F32)
                    eq2 = spool.tile([P, NT], F32)
                    nc.vector.tensor_tensor(out=eq1[:], in0=le[:], in1=m1,
                                            op=mybir.AluOpType.is_equal)
                    nc.vector.tensor_tensor(out=eq2[:], in0=le[:], in1=m2,
                                            op=mybir.AluOpType.is_equal)
                    nc.vector.tensor_add(out=mask_slab[:], in0=eq1[:], in1=eq2[:])
                    nc.vector.tensor_mul(out=comb_slab[:], in0=mask_slab[:], in1=sig[:])

                    # ---- compaction ranks (exclusive prefix of mask in token order) ----
                    csum_ps = rcpsum.tile([1, NT], F32, tag="c1")
                    nc.tensor.matmul(csum_ps[:], lhsT=ones_col[:], rhs=mask_slab[:],
                                     start=True, stop=True)
                    cs = spool.tile([1, NT], F32)
                    nc.vector.tensor_copy(out=cs[:], in_=csum_ps[:])
                    zer = spool.tile([1, NT], F32)
                    nc.vector.memset(zer[:], 0.0)
                    incl = spool.tile([1, NT], F32)
                    nc.vector.tensor_tensor_scan(out=incl[:], data0=cs[:], data1=zer[:],
                                                 initial=0.0,
                                                 op0=mybir.AluOpType.add,
                                                 op1=mybir.AluOpType.add)
                    cpref_row = spool.tile([1, NT], F32)
                    nc.vector.tensor_sub(out=cpref_row[:], in0=incl[:], in1=cs[:])

                    rank_ps = rcpsum.tile([P, NT], F32, tag="rk")
                    nc.tensor.matmul(rank_ps[:], lhsT=ustrict[:], rhs=mask_slab[:],
                                     start=True, stop=False)
                    nc.tensor.matmul(rank_ps[:], lhsT=ones_row[:], rhs=cpref_row[:],
                                     start=False, stop=True)
                    pad_off = spool.tile([P, NT], F32)
                    nc.vector.tensor_scalar(out=pad_off[:], in0=mask_slab[:],
                                            scalar1=-BIG, scalar2=BIG,
                                            op0=mybir.AluOpType.mult,
                                            op1=mybir.AluOpType.add)
                    rank_f = spool.tile([P, NT], F32)
                    nc.vector.tensor_add(out=rank_f[:], in0=rank_ps[:], in1=pad_off[:])
                    nc.vector.tensor_copy(out=rank_i[:], in_=rank_f[:])

                # -------- zero-fill h_c and scat (batched, ACT HWDGE ring) --------
                ZB = 4
                for r in range(TCAP // (P * ZB)):
                    nc.scalar.dma_start(
                        out=h_c[r * P * ZB:(r + 1) * P * ZB, :].rearrange(
                            "(a p) w -> p a w", p=P),
                        in_=zrow[:, None, :].to_broadcast([P, ZB, WPAY]))
                NSC = (T + P) // P
                for r in range(NSC // ZB):
                    nc.scalar.dma_start(
                        out=scat[r * P * ZB:(r + 1) * P * ZB, :].rearrange(
                            "(a p) w -> p a w", p=P),
                        in_=zrow[:, None, 0:H].to_broadcast([P, ZB, H]))
                for r in range((NSC // ZB) * ZB, NSC):
                    nc.scalar.dma_start(out=scat[r * P:(r + 1) * P, :], in_=zrow[:, 0:H])

                # -------- payload scatter pass --------
                with tc.tile_pool(name="ppool", bufs=8) as ppool:
                    for i in range(NT):
                        pay = ppool.tile([P, WPAY], F32, tag="pay")
                        nc.sync.dma_start(out=pay[:, 0:H], in_=h_ext[i * P:(i + 1) * P, :])
                        nc.vector.tensor_copy(out=pay[:, H:H + 1], in_=comb_slab[:, i:i + 1])
                        nc.vector.tensor_copy(out=pay[:, H + 1:H + 2], in_=tok_slab[:, i:i + 1])
                        nc.gpsimd.indirect_dma_start(
                            out=h_c[:],
                            out_offset=bass.IndirectOffsetOnAxis(ap=rank_i[:, i:i + 1], axis=0),
                            in_=pay[:], in_offset=None,
                            bounds_check=TCAP - 1, oob_is_err=False)

            # -------- FFN over compact tokens --------
            with tc.tile_pool(name="fpool", bufs=2) as fpool, \
                 tc.tile_pool(name="hcpool", bufs=CT + 2) as hcpool, \
                 tc.tile_pool(name="gpool", bufs=KF) as gpool, \
                 tc.tile_pool(name="w2pool", bufs=KF) as w2pool, \
                 tc.tile_pool(name="opool", bufs=3) as opool, \
                 tc.tile_pool(name="ftrpsum", bufs=1, space="PSUM") as ftrpsum, \
                 tc.tile_pool(name="fpsum", bufs=2, space="PSUM") as fpsum, \
                 tc.tile_pool(name="opsum", bufs=1, space="PSUM") as opsum:

                # w2 resident in bf16
                w2b = []
                for f in range(KF):
                    w2s = fpool.tile([P, H], F32, tag="w2stage")
                    nc.sync.dma_start(out=w2s[:], in_=w2_ext[f * P:(f + 1) * P, :])
                    w2t = w2pool.tile([P, H], BF16, tag="w2b")
                    nc.vector.tensor_copy(out=w2t[:], in_=w2s[:])
                    w2b.append(w2t)

                for c in range(NCH):
                    hcts = []
                    idxs = []
                    for t in range(CT):
                        hct = hcpool.tile([P, WPAY], F32, tag="hc")
                        r0 = c * CH + t * P
                        nc.sync.dma_start(out=hct[:], in_=h_c[r0:r0 + P, :])
                        idx = hcpool.tile([P, 1], I32, tag="idx")
                        nc.vector.tensor_copy(out=idx[:], in_=hct[:, H + 1:H + 2])
                        hcts.append(hct)
                        idxs.append(idx)
                    hTr = fpool.tile([P, KH, CH], F32R, tag="hTr")
                    for t in range(CT):
                        trp = ftrpsum.tile([P, KH, P], F32, tag="ftr")
                        for k in range(KH):
                            nc.tensor.transpose(out=trp[:, k], in_=hcts[t][:, k * P:(k + 1) * P],
                                                identity=ident[:])
                        nc.vector.tensor_copy(out=hTr[:, :, t * P:(t + 1) * P], in_=trp[:])

                    # stage A: G^T tiles [f, tokens]
                    gts = []
                    for f in range(KF):
                        w1s = fpool.tile([P, KH, P], F32R, tag="w1s")
                        nc.sync.dma_start(
                            out=w1s[:],
                            in_=w1_ext[:, f * P:(f + 1) * P].rearrange("(k p) m -> p k m", p=P))
                        w3s = fpool.tile([P, KH, P], F32R, tag="w3s")
                        nc.sync.dma_start(
                            out=w3s[:],
                            in_=w3_ext[:, f * P:(f + 1) * P].rearrange("(k p) m -> p k m", p=P))
                        x1 = fpsum.tile([P, CH], F32, tag="x1")
                        x3 = fpsum.tile([P, CH], F32, tag="x3")
                        for k in range(KH):
                            nc.tensor.matmul(x1[:], lhsT=w1s[:, k], rhs=hTr[:, k],
                                             start=(k == 0), stop=(k == KH - 1))
                        for k in range(KH):
                            nc.tensor.matmul(x3[:], lhsT=w3s[:, k], rhs=hTr[:, k],
                                             start=(k == 0), stop=(k == KH - 1))
                        gate = fpool.tile([P, CH], F32, tag="gate")
                        nc.scalar.activation(gate[:], x1[:], AF.Silu)
                        gt = gpool.tile([P, CH], BF16, tag="G")
                        nc.vector.tensor_mul(out=gt[:], in0=gate[:], in1=x3[:])
                        gts.append(gt)

                    # stage B: out rows, scaled by combine, scattered to scat
                    for t in range(CT):
                        o = opsum.tile([P, H], F32, tag="o")
                        for f in range(KF):
                            for hh in range(NHALF):
                                nc.tensor.matmul(
                                    o[:, hh * HW2:(hh + 1) * HW2],
                                    lhsT=gts[f][:, t * P:(t + 1) * P],
                                    rhs=w2b[f][:, hh * HW2:(hh + 1) * HW2],
                                    start=(f == 0), stop=(f == KF - 1))
                        osb = opool.tile([P, H], F32, tag="osb")
                        nc.vector.tensor_scalar_mul(osb[:], o[:], hcts[t][:, H:H + 1])
                        nc.gpsimd.indirect_dma_start(
                            out=scat[:],
                            out_offset=bass.IndirectOffsetOnAxis(ap=idxs[t][:, 0:1], axis=0),
                            in_=osb[:], in_offset=None,
                            bounds_check=T + P - 1, oob_is_err=False)

            # -------- collective + output --------
            with tc.tile_pool(name="oc", bufs=2) as ocpool:
                nc.gpsimd.collective_compute(
                    "ReduceScatter", mybir.AluOpType.add,
                    replica_groups=[list(range(n_cores))],
                    ins=[scat[0:T, :]], outs=[rs_out[:]])
                shard = T // n_cores
                for r in range(shard // P):
                    oct_ = ocpool.tile([P, H], F32, tag="oct")
                    nc.sync.dma_start(out=oct_[:], in_=rs_out[r * P:(r + 1) * P, :])
                    nc.sync.dma_start(out=out_ext[r * P:(r + 1) * P, :], in_=oct_[:])

    nc.finalize()
    return nc


def kernel(hidden_states, gate_w, w1, w3, w2):
    T, H = hidden_states.shape
    E, _, FF = w1.shape
    n_cores = 8
    nc = build_kernel(T=T, H=H, FF=FF, E=E, n_cores=n_cores)
    onehots = np.eye(E, dtype=np.float32)
    in_maps = []
    for e in range(n_cores):
        in_maps.append({
            "h": np.ascontiguousarray(hidden_states, dtype=np.float32),
            "gate_w": np.ascontiguousarray(gate_w, dtype=np.float32),
            "w1l": np.ascontiguousarray(w1[e], dtype=np.float32),
            "w3l": np.ascontiguousarray(w3[e], dtype=np.float32),
            "w2l": np.ascontiguousarray(w2[e], dtype=np.float32),
            "onehot": np.tile(onehots[e], (128, 1)),
        })
    res = run_bass_kernel_spmd(nc, in_maps, list(range(n_cores))).results
    return np.concatenate([res[i]["out_shard"] for i in range(n_cores)], axis=0)


if __name__ == "__main__":
    nc = build_kernel()
    print("built", len(nc.inst_map), "instructions")


# revision 9
# speedup vs baseline: 1.1839x; 1.0619x over previous
"""Mixtral MoE layer (top-2 of 8 experts) on 8 Trainium2 NeuronCores.

Strategy: expert parallelism. Core e owns expert e's weights (w1/w3/w2[e]).
Each core:
  1. Router (exact fp32): logits = h @ gate_w, top-2 via max8, combine weight
     for own expert via sigmoid(l_e - l_other); builds a compaction rank for
     the tokens routed to this expert (matmul-based prefix sums).
  2. Compaction: payload rows [h | combine | token_id] are indirect-DMA
     scattered into a dense per-expert buffer h_c (capacity TCAP).
  3. FFN over compact tokens (fp32r stage A, bf16 stage B), scaled by the
     combine weight, indirect-scattered to the token's row of a [T,H] buffer.
  4. ReduceScatter(add) across the 8 cores; host concatenates the shards.
"""
import sys

sys.path.insert(0, "/opt/trn_rl_repo")

import numpy as np

import concourse.bass as bass
import concourse.mybir as mybir
from concourse import bacc
from concourse.tile import TileContext
from concourse.masks import make_identity
from concourse.bass_utils import run_bass_kernel_spmd

F32 = mybir.dt.float32
F32R = mybir.dt.float32r
BF16 = mybir.dt.bfloat16
I32 = mybir.dt.int32
AF = mybir.ActivationFunctionType
P = 128


def build_kernel(T=16384, H=1024, FF=3584, E=8, TCAP=4608, CH=512, n_cores=8):
    NT = T // P      # token tiles
    KH = H // P      # contraction tiles over H
    KF = FF // P     # f tiles (stage A output tiles / stage B contraction)
    NCH = TCAP // CH
    CT = CH // P     # token tiles per FFN chunk
    WPAY = H + 8     # payload row: h | combine | token_id | pad
    TRASH = float(T)  # scatter row for capacity-pad slots
    BIG = 1.0e9
    NHALF = max(1, H // 512)  # stage B free-dim chunks
    HW2 = H // NHALF

    nc = bacc.Bacc(num_devices=n_cores)

    h_ext = nc.dram_tensor("h", [T, H], F32, kind="ExternalInput")
    gw_ext = nc.dram_tensor("gate_w", [H, E], F32, kind="ExternalInput")
    w1_ext = nc.dram_tensor("w1l", [H, FF], F32R, kind="ExternalInput")
    w3_ext = nc.dram_tensor("w3l", [H, FF], F32R, kind="ExternalInput")
    w2_ext = nc.dram_tensor("w2l", [FF, H], F32, kind="ExternalInput")
    oh_ext = nc.dram_tensor("onehot", [P, E], F32, kind="ExternalInput")
    out_ext = nc.dram_tensor("out_shard", [T // n_cores, H], F32, kind="ExternalOutput")

    h_c = nc.dram_tensor("h_c", [TCAP, WPAY], F32)
    scat = nc.dram_tensor("scat", [T + P, H], BF16)
    rs_out = nc.dram_tensor("rs_out", [T // n_cores, H], BF16)

    tok_ids = np.arange(T, dtype=np.float32).reshape(NT, P).T.copy()  # [P, NT]
    tok_const = nc.inline_tensor(tok_ids, name="tok_ids")
    ustrict_np = np.triu(np.ones((P, P), dtype=np.float32), 1)  # [k, m] = 1 iff k < m
    ustrict_const = nc.inline_tensor(ustrict_np, name="ustrict")

    with TileContext(nc) as tc:
        with tc.tile_pool(name="const", bufs=1) as cpool:
            ident = cpool.tile([P, P], F32)
            make_identity(nc, ident[:])
            ustrict = cpool.tile([P, P], F32)
            nc.sync.dma_start(out=ustrict[:], in_=ustrict_const[:])
            tok_slab = cpool.tile([P, NT], F32)
            nc.sync.dma_start(out=tok_slab[:], in_=tok_const[:])
            ones_col = cpool.tile([P, 1], F32)
            nc.vector.memset(ones_col[:], 1.0)
            ones_row = cpool.tile([1, P], F32)
            nc.vector.memset(ones_row[:], 1.0)
            gw_sb = cpool.tile([P, KH, E], F32)
            nc.sync.dma_start(out=gw_sb[:], in_=gw_ext[:].rearrange("(k p) e -> p k e", p=P))
            oh_sb = cpool.tile([P, E], F32)
            nc.sync.dma_start(out=oh_sb[:], in_=oh_ext[:])
            zrow = cpool.tile([P, WPAY], F32)
            nc.vector.memset(zrow[:], 0.0)
            nc.vector.memset(zrow[:, H + 1:H + 2], TRASH)
            zrow_b = cpool.tile([P, H], BF16)
            nc.vector.memset(zrow_b[:], 0.0)

            # -------- router + compaction slabs --------
            with tc.tile_pool(name="rslab", bufs=1) as spool:
                lg_slab = spool.tile([P, NT, E], F32)
                mx_slab = spool.tile([P, NT, 8], F32)
                mask_slab = spool.tile([P, NT], F32)
                comb_slab = spool.tile([P, NT], F32)
                rank_i = spool.tile([P, NT], I32)

                with tc.tile_pool(name="rtile", bufs=3) as rpool, \
                     tc.tile_pool(name="rpsum", bufs=2, space="PSUM") as rpsum, \
                     tc.tile_pool(name="rcpsum", bufs=1, space="PSUM") as rcpsum:
                    for i in range(NT):
                        ht = rpool.tile([P, H], F32, tag="ht")
                        nc.sync.dma_start(out=ht[:], in_=h_ext[i * P:(i + 1) * P, :])
                        trp = rpsum.tile([P, KH, P], F32, tag="trp")
                        for k in range(KH):
                            nc.tensor.transpose(out=trp[:, k], in_=ht[:, k * P:(k + 1) * P],
                                                identity=ident[:])
                        hTt = rpool.tile([P, KH, P], F32, tag="hT")
                        if i % 2 == 0:
                            nc.vector.tensor_copy(out=hTt[:], in_=trp[:])
                        else:
                            nc.scalar.copy(out=hTt[:], in_=trp[:])
                        lg = rpsum.tile([P, E], F32, tag="lg")
                        for k in range(KH):
                            nc.tensor.matmul(lg[:], lhsT=hTt[:, k], rhs=gw_sb[:, k],
                                             start=(k == 0), stop=(k == KH - 1))
                        nc.scalar.copy(out=lg_slab[:, i], in_=lg[:])
                        nc.vector.max(out=mx_slab[:, i], in_=lg_slab[:, i])

                    # batched combine/mask over the full slabs
                    tmp_le = spool.tile([P, NT, E], F32)
                    nc.vector.tensor_mul(out=tmp_le[:], in0=lg_slab[:],
                                          in1=oh_sb[:, None, :].to_broadcast([P, NT, E]))
                    le = spool.tile([P, NT], F32)
                    nc.vector.tensor_reduce(out=le[:], in_=tmp_le[:],
                                            axis=mybir.AxisListType.X,
                                            op=mybir.AluOpType.add)
                    m1 = mx_slab[:, :, 0]
                    m2 = mx_slab[:, :, 1]
                    msum = spool.tile([P, NT], F32)
                    nc.vector.tensor_add(out=msum[:], in0=m1, in1=m2)
                    sgin = spool.tile([P, NT], F32)
                    nc.vector.tensor_scalar_mul(sgin[:], le[:], 2.0)
                    nc.vector.tensor_sub(out=sgin[:], in0=sgin[:], in1=msum[:])
                    sig = spool.tile([P, NT], F32)
                    nc.scalar.activation(sig[:], sgin[:], AF.Sigmoid)
                    eq1 = spool.tile([P, NT], F32)
                    eq2 = spool.tile([P, NT], F32)
                    nc.vector.tensor_tensor(out=eq1[:], in0=le[:], in1=m1,
                                            op=mybir.AluOpType.is_equal)
                    nc.vector.tensor_tensor(out=eq2[:], in0=le[:], in1=m2,
                                            op=mybir.AluOpType.is_equal)
                    nc.vector.tensor_add(out=mask_slab[:], in0=eq1[:], in1=eq2[:])
                    nc.vector.tensor_mul(out=comb_slab[:], in0=mask_slab[:], in1=sig[:])

                    # ---- compaction ranks (exclusive prefix of mask in token order) ----
                    csum_ps = rcpsum.tile([1, NT], F32, tag="c1")
                    nc.tensor.matmul(csum_ps[:], lhsT=ones_col[:], rhs=mask_slab[:],
                                     start=True, stop=True)
                    cs = spool.tile([1, NT], F32)
                    nc.vector.tensor_copy(out=cs[:], in_=csum_ps[:])
                    zer = spool.tile([1, NT], F32)
                    nc.vector.memset(zer[:], 0.0)
                    incl = spool.tile([1, NT], F32)
                    nc.vector.tensor_tensor_scan(out=incl[:], data0=cs[:], data1=zer[:],
                                                 initial=0.0,
                                                 op0=mybir.AluOpType.add,
                                                 op1=mybir.AluOpType.add)
                    cpref_row = spool.tile([1, NT], F32)
                    nc.vector.tensor_sub(out=cpref_row[:], in0=incl[:], in1=cs[:])

                    rank_ps = rcpsum.tile([P, NT], F32, tag="rk")
                    nc.tensor.matmul(rank_ps[:], lhsT=ustrict[:], rhs=mask_slab[:],
                                     start=True, stop=False)
                    nc.tensor.matmul(rank_ps[:], lhsT=ones_row[:], rhs=cpref_row[:],
                                     start=False, stop=True)
                    pad_off = spool.tile([P, NT], F32)
                    nc.vector.tensor_scalar(out=pad_off[:], in0=mask_slab[:],
                                            scalar1=-BIG, scalar2=BIG,
                                            op0=mybir.AluOpType.mult,
                                            op1=mybir.AluOpType.add)
                    rank_f = spool.tile([P, NT], F32)
                    nc.vector.tensor_add(out=rank_f[:], in0=rank_ps[:], in1=pad_off[:])
                    nc.vector.tensor_copy(out=rank_i[:], in_=rank_f[:])

                # -------- zero-fill h_c and scat (batched, ACT HWDGE ring) --------
                ZB = 4
                for r in range(TCAP // (P * ZB)):
                    nc.scalar.dma_start(
                        out=h_c[r * P * ZB:(r + 1) * P * ZB, :].rearrange(
                            "(a p) w -> p a w", p=P),
                        in_=zrow[:, None, :].to_broadcast([P, ZB, WPAY]))
                NSC = (T + P) // P
                for r in range(NSC // ZB):
                    nc.scalar.dma_start(
                        out=scat[r * P * ZB:(r + 1) * P * ZB, :].rearrange(
                            "(a p) w -> p a w", p=P),
                        in_=zrow_b[:, None, :].to_broadcast([P, ZB, H]))
                for r in range((NSC // ZB) * ZB, NSC):
                    nc.scalar.dma_start(out=scat[r * P:(r + 1) * P, :], in_=zrow_b[:])

                # -------- payload scatter pass --------
                with tc.tile_pool(name="ppool", bufs=8) as ppool:
                    for i in range(NT):
                        pay = ppool.tile([P, WPAY], F32, tag="pay")
                        eng = nc.sync if i % 2 == 0 else nc.scalar
                        eng.dma_start(out=pay[:, 0:H], in_=h_ext[i * P:(i + 1) * P, :])
                        nc.vector.tensor_copy(out=pay[:, H:H + 1], in_=comb_slab[:, i:i + 1])
                        nc.vector.tensor_copy(out=pay[:, H + 1:H + 2], in_=tok_slab[:, i:i + 1])
                        nc.gpsimd.indirect_dma_start(
                            out=h_c[:],
                            out_offset=bass.IndirectOffsetOnAxis(ap=rank_i[:, i:i + 1], axis=0),
                            in_=pay[:], in_offset=None,
                            bounds_check=TCAP - 1, oob_is_err=False)

            # -------- FFN over compact tokens --------
            with tc.tile_pool(name="fpool", bufs=2) as fpool, \
                 tc.tile_pool(name="hcpool", bufs=CT + 2) as hcpool, \
                 tc.tile_pool(name="gpool", bufs=KF) as gpool, \
                 tc.tile_pool(name="w2pool", bufs=KF) as w2pool, \
                 tc.tile_pool(name="opool", bufs=3) as opool, \
                 tc.tile_pool(name="ftrpsum", bufs=1, space="PSUM") as ftrpsum, \
                 tc.tile_pool(name="fpsum", bufs=2, space="PSUM") as fpsum, \
                 tc.tile_pool(name="opsum", bufs=1, space="PSUM") as opsum:

                # w2 resident in bf16
                w2b = []
                for f in range(KF):
                    w2s = fpool.tile([P, H], F32, tag="w2stage")
                    nc.sync.dma_start(out=w2s[:], in_=w2_ext[f * P:(f + 1) * P, :])
                    w2t = w2pool.tile([P, H], BF16, tag="w2b")
                    nc.vector.tensor_copy(out=w2t[:], in_=w2s[:])
                    w2b.append(w2t)

                for c in range(NCH):
                    hcts = []
                    idxs = []
                    for t in range(CT):
                        hct = hcpool.tile([P, WPAY], F32, tag="hc")
                        r0 = c * CH + t * P
                        nc.sync.dma_start(out=hct[:], in_=h_c[r0:r0 + P, :])
                        idx = hcpool.tile([P, 1], I32, tag="idx")
                        nc.vector.tensor_copy(out=idx[:], in_=hct[:, H + 1:H + 2])
                        hcts.append(hct)
                        idxs.append(idx)
                    hTr = fpool.tile([P, KH, CH], F32R, tag="hTr")
                    for t in range(CT):
                        trp = ftrpsum.tile([P, KH, P], F32, tag="ftr")
                        for k in range(KH):
                            nc.tensor.transpose(out=trp[:, k], in_=hcts[t][:, k * P:(k + 1) * P],
                                                identity=ident[:])
                        nc.vector.tensor_copy(out=hTr[:, :, t * P:(t + 1) * P], in_=trp[:])

                    # stage A: G^T tiles [f, tokens]
                    gts = []
                    for f in range(KF):
                        w1s = fpool.tile([P, KH, P], F32R, tag="w1s")
                        nc.sync.dma_start(
                            out=w1s[:],
                            in_=w1_ext[:, f * P:(f + 1) * P].rearrange("(k p) m -> p k m", p=P))
                        w3s = fpool.tile([P, KH, P], F32R, tag="w3s")
                        nc.sync.dma_start(
                            out=w3s[:],
                            in_=w3_ext[:, f * P:(f + 1) * P].rearrange("(k p) m -> p k m", p=P))
                        x1 = fpsum.tile([P, CH], F32, tag="x1")
                        x3 = fpsum.tile([P, CH], F32, tag="x3")
                        for k in range(KH):
                            nc.tensor.matmul(x1[:], lhsT=w1s[:, k], rhs=hTr[:, k],
                                             start=(k == 0), stop=(k == KH - 1))
                        for k in range(KH):
                            nc.tensor.matmul(x3[:], lhsT=w3s[:, k], rhs=hTr[:, k],
                                             start=(k == 0), stop=(k == KH - 1))
                        gate = fpool.tile([P, CH], F32, tag="gate")
                        nc.scalar.activation(gate[:], x1[:], AF.Silu)
                        gt = gpool.tile([P, CH], BF16, tag="G")
                        nc.vector.tensor_mul(out=gt[:], in0=gate[:], in1=x3[:])
                        gts.append(gt)

                    # stage B: out rows, scaled by combine, scattered to scat
                    for t in range(CT):
                        o = opsum.tile([P, H], F32, tag="o")
                        for f in range(KF):
                            for hh in range(NHALF):
                                nc.tensor.matmul(
                                    o[:, hh * HW2:(hh + 1) * HW2],
                                    lhsT=gts[f][:, t * P:(t + 1) * P],
                                    rhs=w2b[f][:, hh * HW2:(hh + 1) * HW2],
                                    start=(f == 0), stop=(f == KF - 1))
                        osb = opool.tile([P, H], BF16, tag="osb")
                        nc.vector.tensor_scalar_mul(osb[:], o[:], hcts[t][:, H:H + 1])
                        nc.gpsimd.indirect_dma_start(
                            out=scat[:],
                            out_offset=bass.IndirectOffsetOnAxis(ap=idxs[t][:, 0:1], axis=0),
                            in_=osb[:], in_offset=None,
                            bounds_check=T + P - 1, oob_is_err=False)

            # -------- collective + output --------
            with tc.tile_pool(name="oc", bufs=2) as ocpool:
                nc.gpsimd.collective_compute(
                    "ReduceScatter", mybir.AluOpType.add,
                    replica_groups=[list(range(n_cores))],
                    ins=[scat[0:T, :]], outs=[rs_out[:]])
                shard = T // n_cores
                for r in range(shard // P):
                    oct_ = ocpool.tile([P, H], BF16, tag="oct")
                    nc.sync.dma_start(out=oct_[:], in_=rs_out[r * P:(r + 1) * P, :])
                    octf = ocpool.tile([P, H], F32, tag="octf")
                    nc.vector.tensor_copy(out=octf[:], in_=oct_[:])
                    nc.sync.dma_start(out=out_ext[r * P:(r + 1) * P, :], in_=octf[:])

    nc.finalize()
    return nc


def kernel(hidden_states, gate_w, w1, w3, w2):
    T, H = hidden_states.shape
    E, _, FF = w1.shape
    n_cores = 8
    nc = build_kernel(T=T, H=H, FF=FF, E=E, n_cores=n_cores)
    onehots = np.eye(E, dtype=np.float32)
    in_maps = []
    for e in range(n_cores):
        in_maps.append({
            "h": np.ascontiguousarray(hidden_states, dtype=np.float32),
            "gate_w": np.ascontiguousarray(gate_w, dtype=np.float32),
            "w1l": np.ascontiguousarray(w1[e], dtype=np.float32),
            "w3l": np.ascontiguousarray(w3[e], dtype=np.float32),
            "w2l": np.ascontiguousarray(w2[e], dtype=np.float32),
            "onehot": np.tile(onehots[e], (128, 1)),
        })
    res = run_bass_kernel_spmd(nc, in_maps, list(range(n_cores))).results
    return np.concatenate([res[i]["out_shard"] for i in range(n_cores)], axis=0)


if __name__ == "__main__":
    nc = build_kernel()
    print("built", len(nc.inst_map), "instructions")


# revision 12
# speedup vs baseline: 1.1973x; 1.0113x over previous
"""Mixtral MoE layer (top-2 of 8 experts) on 8 Trainium2 NeuronCores.

Strategy: expert parallelism. Core e owns expert e's weights (w1/w3/w2[e]).
Each core:
  1. Router (exact fp32): logits = h @ gate_w, top-2 via max8, combine weight
     for own expert via sigmoid(l_e - l_other); builds a compaction rank for
     the tokens routed to this expert (matmul-based prefix sums).
  2. Compaction: payload rows [h | combine | token_id] are indirect-DMA
     scattered into a dense per-expert buffer h_c (capacity TCAP).
  3. FFN over compact tokens (fp32r stage A, bf16 stage B), scaled by the
     combine weight, indirect-scattered to the token's row of a [T,H] buffer.
  4. ReduceScatter(add) across the 8 cores; host concatenates the shards.
"""
import sys

sys.path.insert(0, "/opt/trn_rl_repo")

import numpy as np

import concourse.bass as bass
import concourse.mybir as mybir
from concourse import bacc
from concourse.tile import TileContext
from concourse.masks import make_identity
from concourse.bass_utils import run_bass_kernel_spmd

F32 = mybir.dt.float32
F32R = mybir.dt.float32r
BF16 = mybir.dt.bfloat16
I32 = mybir.dt.int32
AF = mybir.ActivationFunctionType
P = 128


def build_kernel(T=16384, H=1024, FF=3584, E=8, TCAP=4608, CH=512, n_cores=8):
    NT = T // P      # token tiles
    KH = H // P      # contraction tiles over H
    KF = FF // P     # f tiles (stage A output tiles / stage B contraction)
    NCH = TCAP // CH
    CT = CH // P     # token tiles per FFN chunk
    WPAY = H + 8     # payload row: h | combine | token_id | pad
    TRASH = float(T)  # scatter row for capacity-pad slots
    BIG = 1.0e9
    NHALF = max(1, H // 512)  # stage B free-dim chunks
    HW2 = H // NHALF

    nc = bacc.Bacc(num_devices=n_cores, num_swdge_queues=4)

    h_ext = nc.dram_tensor("h", [T, H], F32, kind="ExternalInput")
    gw_ext = nc.dram_tensor("gate_w", [H, E], F32, kind="ExternalInput")
    w1_ext = nc.dram_tensor("w1l", [H, FF], F32R, kind="ExternalInput")
    w3_ext = nc.dram_tensor("w3l", [H, FF], F32R, kind="ExternalInput")
    w2_ext = nc.dram_tensor("w2l", [FF, H], F32, kind="ExternalInput")
    oh_ext = nc.dram_tensor("onehot", [P, E], F32, kind="ExternalInput")
    out_ext = nc.dram_tensor("out_shard", [T // n_cores, H], F32, kind="ExternalOutput")

    h_c = nc.dram_tensor("h_c", [TCAP, WPAY], F32)
    scat = nc.dram_tensor("scat", [T + P, H], BF16)
    rs_out = nc.dram_tensor("rs_out", [T // n_cores, H], BF16)

    tok_ids = np.arange(T, dtype=np.float32).reshape(NT, P).T.copy()  # [P, NT]
    tok_const = nc.inline_tensor(tok_ids, name="tok_ids")
    ustrict_np = np.triu(np.ones((P, P), dtype=np.float32), 1)  # [k, m] = 1 iff k < m
    ustrict_const = nc.inline_tensor(ustrict_np, name="ustrict")

    with TileContext(nc) as tc:
        with tc.tile_pool(name="const", bufs=1) as cpool:
            ident = cpool.tile([P, P], F32)
            make_identity(nc, ident[:])
            ustrict = cpool.tile([P, P], F32)
            nc.sync.dma_start(out=ustrict[:], in_=ustrict_const[:])
            tok_slab = cpool.tile([P, NT], F32)
            nc.sync.dma_start(out=tok_slab[:], in_=tok_const[:])
            ones_col = cpool.tile([P, 1], F32)
            nc.vector.memset(ones_col[:], 1.0)
            ones_row = cpool.tile([1, P], F32)
            nc.vector.memset(ones_row[:], 1.0)
            gw_sb = cpool.tile([P, KH, E], F32)
            nc.sync.dma_start(out=gw_sb[:], in_=gw_ext[:].rearrange("(k p) e -> p k e", p=P))
            oh_sb = cpool.tile([P, E], F32)
            nc.sync.dma_start(out=oh_sb[:], in_=oh_ext[:])
            zrow = cpool.tile([P, WPAY], F32)
            nc.vector.memset(zrow[:], 0.0)
            nc.vector.memset(zrow[:, H + 1:H + 2], TRASH)
            zrow_b = cpool.tile([P, H], BF16)
            nc.vector.memset(zrow_b[:], 0.0)

            # -------- router + compaction slabs --------
            with tc.tile_pool(name="rslab", bufs=1) as spool:
                lg_slab = spool.tile([P, NT, E], F32)
                mx_slab = spool.tile([P, NT, 8], F32)
                mask_slab = spool.tile([P, NT], F32)
                comb_slab = spool.tile([P, NT], F32)
                rank_i = spool.tile([P, NT], I32)

                with tc.tile_pool(name="rtile", bufs=3) as rpool, \
                     tc.tile_pool(name="rpsum", bufs=2, space="PSUM") as rpsum, \
                     tc.tile_pool(name="rcpsum", bufs=1, space="PSUM") as rcpsum:
                    for i in range(NT):
                        ht = rpool.tile([P, H], F32, tag="ht")
                        heng = nc.sync if i % 2 == 0 else nc.scalar
                        heng.dma_start(out=ht[:], in_=h_ext[i * P:(i + 1) * P, :])
                        trp = rpsum.tile([P, KH, P], F32, tag="trp")
                        for k in range(KH):
                            nc.tensor.transpose(out=trp[:, k], in_=ht[:, k * P:(k + 1) * P],
                                                identity=ident[:])
                        hTt = rpool.tile([P, KH, P], F32, tag="hT")
                        if i % 2 == 0:
                            nc.vector.tensor_copy(out=hTt[:], in_=trp[:])
                        else:
                            nc.scalar.copy(out=hTt[:], in_=trp[:])
                        lg = rpsum.tile([P, E], F32, tag="lg")
                        for k in range(KH):
                            nc.tensor.matmul(lg[:], lhsT=hTt[:, k], rhs=gw_sb[:, k],
                                             start=(k == 0), stop=(k == KH - 1))
                        nc.scalar.copy(out=lg_slab[:, i], in_=lg[:])
                        nc.vector.max(out=mx_slab[:, i], in_=lg_slab[:, i])

                    # batched combine/mask over the full slabs
                    tmp_le = spool.tile([P, NT, E], F32)
                    nc.vector.tensor_mul(out=tmp_le[:], in0=lg_slab[:],
                                          in1=oh_sb[:, None, :].to_broadcast([P, NT, E]))
                    le = spool.tile([P, NT], F32)
                    nc.vector.tensor_reduce(out=le[:], in_=tmp_le[:],
                                            axis=mybir.AxisListType.X,
                                            op=mybir.AluOpType.add)
                    m1 = mx_slab[:, :, 0]
                    m2 = mx_slab[:, :, 1]
                    msum = spool.tile([P, NT], F32)
                    nc.vector.tensor_add(out=msum[:], in0=m1, in1=m2)
                    sgin = spool.tile([P, NT], F32)
                    nc.vector.tensor_scalar_mul(sgin[:], le[:], 2.0)
                    nc.vector.tensor_sub(out=sgin[:], in0=sgin[:], in1=msum[:])
                    sig = spool.tile([P, NT], F32)
                    nc.scalar.activation(sig[:], sgin[:], AF.Sigmoid)
                    eq1 = spool.tile([P, NT], F32)
                    eq2 = spool.tile([P, NT], F32)
                    nc.vector.tensor_tensor(out=eq1[:], in0=le[:], in1=m1,
                                            op=mybir.AluOpType.is_equal)
                    nc.vector.tensor_tensor(out=eq2[:], in0=le[:], in1=m2,
                                            op=mybir.AluOpType.is_equal)
                    nc.vector.tensor_add(out=mask_slab[:], in0=eq1[:], in1=eq2[:])
                    nc.vector.tensor_mul(out=comb_slab[:], in0=mask_slab[:], in1=sig[:])

                    # ---- compaction ranks (exclusive prefix of mask in token order) ----
                    csum_ps = rcpsum.tile([1, NT], F32, tag="c1")
                    nc.tensor.matmul(csum_ps[:], lhsT=ones_col[:], rhs=mask_slab[:],
                                     start=True, stop=True)
                    cs = spool.tile([1, NT], F32)
                    nc.vector.tensor_copy(out=cs[:], in_=csum_ps[:])
                    zer = spool.tile([1, NT], F32)
                    nc.vector.memset(zer[:], 0.0)
                    incl = spool.tile([1, NT], F32)
                    nc.vector.tensor_tensor_scan(out=incl[:], data0=cs[:], data1=zer[:],
                                                 initial=0.0,
                                                 op0=mybir.AluOpType.add,
                                                 op1=mybir.AluOpType.add)
                    cpref_row = spool.tile([1, NT], F32)
                    nc.vector.tensor_sub(out=cpref_row[:], in0=incl[:], in1=cs[:])

                    rank_ps = rcpsum.tile([P, NT], F32, tag="rk")
                    nc.tensor.matmul(rank_ps[:], lhsT=ustrict[:], rhs=mask_slab[:],
                                     start=True, stop=False)
                    nc.tensor.matmul(rank_ps[:], lhsT=ones_row[:], rhs=cpref_row[:],
                                     start=False, stop=True)
                    pad_off = spool.tile([P, NT], F32)
                    nc.vector.tensor_scalar(out=pad_off[:], in0=mask_slab[:],
                                            scalar1=-BIG, scalar2=BIG,
                                            op0=mybir.AluOpType.mult,
                                            op1=mybir.AluOpType.add)
                    rank_f = spool.tile([P, NT], F32)
                    nc.vector.tensor_add(out=rank_f[:], in0=rank_ps[:], in1=pad_off[:])
                    nc.vector.tensor_copy(out=rank_i[:], in_=rank_f[:])

                # -------- zero-fill h_c and scat (batched, ACT HWDGE ring) --------
                ZB = 4
                for r in range(TCAP // (P * ZB)):
                    nc.scalar.dma_start(
                        out=h_c[r * P * ZB:(r + 1) * P * ZB, :].rearrange(
                            "(a p) w -> p a w", p=P),
                        in_=zrow[:, None, :].to_broadcast([P, ZB, WPAY]))
                NSC = (T + P) // P
                for r in range(NSC // ZB):
                    nc.scalar.dma_start(
                        out=scat[r * P * ZB:(r + 1) * P * ZB, :].rearrange(
                            "(a p) w -> p a w", p=P),
                        in_=zrow_b[:, None, :].to_broadcast([P, ZB, H]))
                for r in range((NSC // ZB) * ZB, NSC):
                    nc.scalar.dma_start(out=scat[r * P:(r + 1) * P, :], in_=zrow_b[:])

                # -------- payload scatter pass --------
                with tc.tile_pool(name="ppool", bufs=20) as ppool:
                    for i in range(NT):
                        pay = ppool.tile([P, WPAY], F32, tag="pay")
                        eng = nc.sync if i % 2 == 0 else nc.scalar
                        eng.dma_start(out=pay[:, 0:H], in_=h_ext[i * P:(i + 1) * P, :])
                        nc.vector.tensor_copy(out=pay[:, H:H + 1], in_=comb_slab[:, i:i + 1])
                        nc.vector.tensor_copy(out=pay[:, H + 1:H + 2], in_=tok_slab[:, i:i + 1])
                        nc.gpsimd.indirect_dma_start(
                            out=h_c[:],
                            out_offset=bass.IndirectOffsetOnAxis(ap=rank_i[:, i:i + 1], axis=0),
                            in_=pay[:], in_offset=None,
                            bounds_check=TCAP - 1, oob_is_err=False)

            # -------- FFN over compact tokens --------
            with tc.tile_pool(name="fpool", bufs=2) as fpool, \
                 tc.tile_pool(name="wpool", bufs=4) as wpool, \
                 tc.tile_pool(name="hcpool", bufs=CT + 2) as hcpool, \
                 tc.tile_pool(name="gpool", bufs=KF) as gpool, \
                 tc.tile_pool(name="w2pool", bufs=KF) as w2pool, \
                 tc.tile_pool(name="opool", bufs=3) as opool, \
                 tc.tile_pool(name="ftrpsum", bufs=1, space="PSUM") as ftrpsum, \
                 tc.tile_pool(name="fpsum", bufs=2, space="PSUM") as fpsum, \
                 tc.tile_pool(name="opsum", bufs=1, space="PSUM") as opsum:

                # w2 resident in bf16
                w2b = []
                for f in range(KF):
                    w2s = fpool.tile([P, H], F32, tag="w2stage")
                    nc.sync.dma_start(out=w2s[:], in_=w2_ext[f * P:(f + 1) * P, :])
                    w2t = w2pool.tile([P, H], BF16, tag="w2b")
                    nc.vector.tensor_copy(out=w2t[:], in_=w2s[:])
                    w2b.append(w2t)

                for c in range(NCH):
                    hcts = []
                    idxs = []
                    for t in range(CT):
                        hct = hcpool.tile([P, WPAY], F32, tag="hc")
                        r0 = c * CH + t * P
                        nc.sync.dma_start(out=hct[:], in_=h_c[r0:r0 + P, :])
                        idx = hcpool.tile([P, 1], I32, tag="idx")
                        nc.vector.tensor_copy(out=idx[:], in_=hct[:, H + 1:H + 2])
                        hcts.append(hct)
                        idxs.append(idx)
                    hTr = fpool.tile([P, KH, CH], F32R, tag="hTr")
                    for t in range(CT):
                        trp = ftrpsum.tile([P, KH, P], F32, tag="ftr")
                        for k in range(KH):
                            nc.tensor.transpose(out=trp[:, k], in_=hcts[t][:, k * P:(k + 1) * P],
                                                identity=ident[:])
                        nc.vector.tensor_copy(out=hTr[:, :, t * P:(t + 1) * P], in_=trp[:])

                    # stage A: G^T tiles [f, tokens]
                    gts = []
                    for f in range(KF):
                        w1s = wpool.tile([P, KH, P], F32R, tag="w1s")
                        nc.sync.dma_start(
                            out=w1s[:],
                            in_=w1_ext[:, f * P:(f + 1) * P].rearrange("(k p) m -> p k m", p=P))
                        w3s = wpool.tile([P, KH, P], F32R, tag="w3s")
                        nc.sync.dma_start(
                            out=w3s[:],
                            in_=w3_ext[:, f * P:(f + 1) * P].rearrange("(k p) m -> p k m", p=P))
                        x1 = fpsum.tile([P, CH], F32, tag="x1")
                        x3 = fpsum.tile([P, CH], F32, tag="x3")
                        for k in range(KH):
                            nc.tensor.matmul(x1[:], lhsT=w1s[:, k], rhs=hTr[:, k],
                                             start=(k == 0), stop=(k == KH - 1))
                        for k in range(KH):
                            nc.tensor.matmul(x3[:], lhsT=w3s[:, k], rhs=hTr[:, k],
                                             start=(k == 0), stop=(k == KH - 1))
                        gate = fpool.tile([P, CH], F32, tag="gate")
                        nc.scalar.activation(gate[:], x1[:], AF.Silu)
                        gt = gpool.tile([P, CH], BF16, tag="G")
                        nc.vector.tensor_mul(out=gt[:], in0=gate[:], in1=x3[:])
                        gts.append(gt)

                    # stage B: out rows, scaled by combine, scattered to scat
                    for t in range(CT):
                        o = opsum.tile([P, H], F32, tag="o")
                        for f in range(KF):
                            for hh in range(NHALF):
                                nc.tensor.matmul(
                                    o[:, hh * HW2:(hh + 1) * HW2],
                                    lhsT=gts[f][:, t * P:(t + 1) * P],
                                    rhs=w2b[f][:, hh * HW2:(hh + 1) * HW2],
                                    start=(f == 0), stop=(f == KF - 1))
                        osb = opool.tile([P, H], BF16, tag="osb")
                        nc.vector.tensor_scalar_mul(osb[:], o[:], hcts[t][:, H:H + 1])
                        nc.gpsimd.indirect_dma_start(
                            out=scat[:],
                            out_offset=bass.IndirectOffsetOnAxis(ap=idxs[t][:, 0:1], axis=0),
                            in_=osb[:], in_offset=None,
                            bounds_check=T + P - 1, oob_is_err=False)

            # -------- collective + output --------
            with tc.tile_pool(name="oc", bufs=2) as ocpool:
                nc.gpsimd.collective_compute(
                    "ReduceScatter", mybir.AluOpType.add,
                    replica_groups=[list(range(n_cores))],
                    ins=[scat[0:T, :]], outs=[rs_out[:]])
                shard = T // n_cores
                for r in range(shard // P):
                    oct_ = ocpool.tile([P, H], BF16, tag="oct")
                    nc.sync.dma_start(out=oct_[:], in_=rs_out[r * P:(r + 1) * P, :])
                    octf = ocpool.tile([P, H], F32, tag="octf")
                    nc.vector.tensor_copy(out=octf[:], in_=oct_[:])
                    nc.sync.dma_start(out=out_ext[r * P:(r + 1) * P, :], in_=octf[:])

    nc.finalize()
    return nc


def kernel(hidden_states, gate_w, w1, w3, w2):
    T, H = hidden_states.shape
    E, _, FF = w1.shape
    n_cores = 8
    nc = build_kernel(T=T, H=H, FF=FF, E=E, n_cores=n_cores)
    onehots = np.eye(E, dtype=np.float32)
    in_maps = []
    for e in range(n_cores):
        in_maps.append({
            "h": np.ascontiguousarray(hidden_states, dtype=np.float32),
            "gate_w": np.ascontiguousarray(gate_w, dtype=np.float32),
            "w1l": np.ascontiguousarray(w1[e], dtype=np.float32),
            "w3l": np.ascontiguousarray(w3[e], dtype=np.float32),
            "w2l": np.ascontiguousarray(w2[e], dtype=np.float32),
            "onehot": np.tile(onehots[e], (128, 1)),
        })
    res = run_bass_kernel_spmd(nc, in_maps, list(range(n_cores))).results
    return np.concatenate([res[i]["out_shard"] for i in range(n_cores)], axis=0)


if __name__ == "__main__":
    nc = build_kernel()
    print("built", len(nc.inst_map), "instructions")


# revision 15
# speedup vs baseline: 1.2200x; 1.0189x over previous
"""Mixtral MoE layer (top-2 of 8 experts) on 8 Trainium2 NeuronCores.

Strategy: expert parallelism. Core e owns expert e's weights (w1/w3/w2[e]).
Each core:
  1. Router (exact fp32): logits = h @ gate_w, top-2 via max8, combine weight
     for own expert via sigmoid(l_e - l_other); builds a compaction rank for
     the tokens routed to this expert (matmul-based prefix sums).
  2. Compaction: payload rows [h | combine | token_id] are indirect-DMA
     scattered into a dense per-expert buffer h_c (capacity TCAP).
  3. FFN over compact tokens (fp32r stage A, bf16 stage B), scaled by the
     combine weight, indirect-scattered to the token's row of a [T,H] buffer.
  4. ReduceScatter(add) across the 8 cores; host concatenates the shards.
"""
import sys

sys.path.insert(0, "/opt/trn_rl_repo")

import numpy as np

import concourse.bass as bass
import concourse.mybir as mybir
from concourse import bacc
from concourse.tile import TileContext
from concourse.masks import make_identity
from concourse.bass_utils import run_bass_kernel_spmd

F32 = mybir.dt.float32
F32R = mybir.dt.float32r
BF16 = mybir.dt.bfloat16
I32 = mybir.dt.int32
AF = mybir.ActivationFunctionType
P = 128


def build_kernel(T=16384, H=1024, FF=3584, E=8, TCAP=4608, CH=512, n_cores=8):
    NT = T // P      # token tiles
    KH = H // P      # contraction tiles over H
    KF = FF // P     # f tiles (stage A output tiles / stage B contraction)
    NCH = TCAP // CH
    CT = CH // P     # token tiles per FFN chunk
    WPAY = H + 8     # payload row: h | combine | token_id | pad
    TRASH = float(T)  # scatter row for capacity-pad slots
    BIG = 1.0e9
    NHALF = max(1, H // 512)  # stage B free-dim chunks
    HW2 = H // NHALF

    nc = bacc.Bacc(num_devices=n_cores, num_swdge_queues=4)

    h_ext = nc.dram_tensor("h", [T, H], F32, kind="ExternalInput")
    gw_ext = nc.dram_tensor("gate_w", [H, E], F32, kind="ExternalInput")
    w1_ext = nc.dram_tensor("w1l", [H, FF], F32R, kind="ExternalInput")
    w3_ext = nc.dram_tensor("w3l", [H, FF], F32R, kind="ExternalInput")
    w2_ext = nc.dram_tensor("w2l", [FF, H], F32, kind="ExternalInput")
    oh_ext = nc.dram_tensor("onehot", [P, E], F32, kind="ExternalInput")
    out_ext = nc.dram_tensor("out_shard", [T // n_cores, H], F32, kind="ExternalOutput")

    h_c = nc.dram_tensor("h_c", [TCAP, WPAY], F32)
    scat = nc.dram_tensor("scat", [T + P, H], BF16)
    rs_out = nc.dram_tensor("rs_out", [T // n_cores, H], BF16)

    tok_ids = np.arange(T, dtype=np.float32).reshape(NT, P).T.copy()  # [P, NT]
    tok_const = nc.inline_tensor(tok_ids, name="tok_ids")
    ustrict_np = np.triu(np.ones((P, P), dtype=np.float32), 1)  # [k, m] = 1 iff k < m
    ustrict_const = nc.inline_tensor(ustrict_np, name="ustrict")

    with TileContext(nc) as tc:
        with tc.tile_pool(name="const", bufs=1) as cpool:
            ident = cpool.tile([P, P], F32)
            make_identity(nc, ident[:])
            ustrict = cpool.tile([P, P], F32)
            nc.sync.dma_start(out=ustrict[:], in_=ustrict_const[:])
            tok_slab = cpool.tile([P, NT], F32)
            nc.sync.dma_start(out=tok_slab[:], in_=tok_const[:])
            ones_col = cpool.tile([P, 1], F32)
            nc.vector.memset(ones_col[:], 1.0)
            ones_row = cpool.tile([1, P], F32)
            nc.vector.memset(ones_row[:], 1.0)
            gw_sb = cpool.tile([P, KH, E], F32)
            nc.sync.dma_start(out=gw_sb[:], in_=gw_ext[:].rearrange("(k p) e -> p k e", p=P))
            oh_sb = cpool.tile([P, E], F32)
            nc.sync.dma_start(out=oh_sb[:], in_=oh_ext[:])
            zrow = cpool.tile([P, WPAY], F32)
            nc.vector.memset(zrow[:], 0.0)
            nc.vector.memset(zrow[:, H + 1:H + 2], TRASH)
            zrow_b = cpool.tile([P, H], BF16)
            nc.vector.memset(zrow_b[:], 0.0)
            zer_row = cpool.tile([1, P], F32)
            nc.vector.memset(zer_row[:], 0.0)

            # -------- router + compaction + payload, in overlapped groups --------
            # Tokens are processed in NG groups of GT tiles. Each group computes
            # its logits/top2/combine, then its compaction ranks; the global rank
            # base is carried between groups by chaining tensor_tensor_scan
            # (initial = previous group's last inclusive prefix). A group's
            # payload scatter only depends on its own ranks, so it overlaps the
            # next group's router compute/DMA instead of serializing at the end.
            # -------- zero-fill h_c and scat (batched, SWDGE queues) --------
            ZB = 4
            for r in range(TCAP // (P * ZB)):
                nc.gpsimd.dma_start(
                    out=h_c[r * P * ZB:(r + 1) * P * ZB, :].rearrange(
                        "(a p) w -> p a w", p=P),
                    in_=zrow[:, None, :].to_broadcast([P, ZB, WPAY]))
            NSC = (T + P) // P
            for r in range(NSC // ZB):
                nc.gpsimd.dma_start(
                    out=scat[r * P * ZB:(r + 1) * P * ZB, :].rearrange(
                        "(a p) w -> p a w", p=P),
                    in_=zrow_b[:, None, :].to_broadcast([P, ZB, H]))
            for r in range((NSC // ZB) * ZB, NSC):
                nc.gpsimd.dma_start(out=scat[r * P:(r + 1) * P, :], in_=zrow_b[:])

            GT = min(16, NT)
            NG = NT // GT
            with tc.tile_pool(name="rslab", bufs=1) as spool:
                mx_slab = spool.tile([P, NT, 8], F32)
                comb_slab = spool.tile([P, NT], F32)
                rank_i = spool.tile([P, NT], I32)
                cs_slab = spool.tile([1, NT], F32)
                incl_slab = spool.tile([1, NT], F32)

                with tc.tile_pool(name="rtile", bufs=3) as rpool, \
                     tc.tile_pool(name="rgrp", bufs=2) as gpool_r, \
                     tc.tile_pool(name="ppool", bufs=8) as ppool, \
                     tc.tile_pool(name="rpsum", bufs=2, space="PSUM") as rpsum, \
                     tc.tile_pool(name="rcpsum", bufs=1, space="PSUM") as rcpsum:
                    for q in range(NG):
                        i0 = q * GT
                        lg_g = gpool_r.tile([P, GT, E], F32, tag="lg_g")
                        for j in range(GT):
                            i = i0 + j
                            ht = rpool.tile([P, H], F32, tag="ht")
                            heng = nc.sync if i % 2 == 0 else nc.scalar
                            heng.dma_start(out=ht[:], in_=h_ext[i * P:(i + 1) * P, :])
                            trp = rpsum.tile([P, KH, P], F32, tag="trp")
                            for k in range(KH):
                                nc.tensor.transpose(out=trp[:, k],
                                                    in_=ht[:, k * P:(k + 1) * P],
                                                    identity=ident[:])
                            hTt = rpool.tile([P, KH, P], F32, tag="hT")
                            if i % 2 == 0:
                                nc.vector.tensor_copy(out=hTt[:], in_=trp[:])
                            else:
                                nc.scalar.copy(out=hTt[:], in_=trp[:])
                            lg = rpsum.tile([P, E], F32, tag="lg")
                            for k in range(KH):
                                nc.tensor.matmul(lg[:], lhsT=hTt[:, k], rhs=gw_sb[:, k],
                                                 start=(k == 0), stop=(k == KH - 1))
                            nc.scalar.copy(out=lg_g[:, j], in_=lg[:])
                            nc.vector.max(out=mx_slab[:, i], in_=lg_g[:, j])

                        # group combine/mask
                        sl = slice(i0, i0 + GT)
                        tmp_le = gpool_r.tile([P, GT, E], F32, tag="tmp_le")
                        nc.vector.tensor_mul(out=tmp_le[:], in0=lg_g[:],
                                             in1=oh_sb[:, None, :].to_broadcast([P, GT, E]))
                        le = gpool_r.tile([P, GT], F32, tag="le")
                        nc.vector.tensor_reduce(out=le[:], in_=tmp_le[:],
                                                axis=mybir.AxisListType.X,
                                                op=mybir.AluOpType.add)
                        m1 = mx_slab[:, sl, 0]
                        m2 = mx_slab[:, sl, 1]
                        msum = gpool_r.tile([P, GT], F32, tag="msum")
                        nc.vector.tensor_add(out=msum[:], in0=m1, in1=m2)
                        sgin = gpool_r.tile([P, GT], F32, tag="sgin")
                        nc.vector.tensor_scalar_mul(sgin[:], le[:], 2.0)
                        nc.vector.tensor_sub(out=sgin[:], in0=sgin[:], in1=msum[:])
                        sig = gpool_r.tile([P, GT], F32, tag="sig")
                        nc.scalar.activation(sig[:], sgin[:], AF.Sigmoid)
                        eq1 = gpool_r.tile([P, GT], F32, tag="eq1")
                        eq2 = gpool_r.tile([P, GT], F32, tag="eq2")
                        nc.vector.tensor_tensor(out=eq1[:], in0=le[:], in1=m1,
                                                op=mybir.AluOpType.is_equal)
                        nc.vector.tensor_tensor(out=eq2[:], in0=le[:], in1=m2,
                                                op=mybir.AluOpType.is_equal)
                        mask_g = gpool_r.tile([P, GT], F32, tag="mask_g")
                        nc.vector.tensor_add(out=mask_g[:], in0=eq1[:], in1=eq2[:])
                        nc.vector.tensor_mul(out=comb_slab[:, sl], in0=mask_g[:], in1=sig[:])

                        # group compaction ranks with chained global base
                        csum_ps = rcpsum.tile([1, GT], F32, tag="c1")
                        nc.tensor.matmul(csum_ps[:], lhsT=ones_col[:], rhs=mask_g[:],
                                         start=True, stop=True)
                        nc.vector.tensor_copy(out=cs_slab[:, sl], in_=csum_ps[:])
                        init = 0.0 if q == 0 else incl_slab[:, i0 - 1:i0]
                        nc.vector.tensor_tensor_scan(out=incl_slab[:, sl],
                                                     data0=cs_slab[:, sl],
                                                     data1=zer_row[:, 0:GT],
                                                     initial=init,
                                                     op0=mybir.AluOpType.add,
                                                     op1=mybir.AluOpType.add)
                        cpref = gpool_r.tile([1, GT], F32, tag="cpref")
                        nc.vector.tensor_sub(out=cpref[:], in0=incl_slab[:, sl],
                                             in1=cs_slab[:, sl])
                        rank_ps = rcpsum.tile([P, GT], F32, tag="rk")
                        nc.tensor.matmul(rank_ps[:], lhsT=ustrict[:], rhs=mask_g[:],
                                         start=True, stop=False)
                        nc.tensor.matmul(rank_ps[:], lhsT=ones_row[:], rhs=cpref[:],
                                         start=False, stop=True)
                        pad_off = gpool_r.tile([P, GT], F32, tag="pad_off")
                        nc.vector.tensor_scalar(out=pad_off[:], in0=mask_g[:],
                                                scalar1=-BIG, scalar2=BIG,
                                                op0=mybir.AluOpType.mult,
                                                op1=mybir.AluOpType.add)
                        rank_f = gpool_r.tile([P, GT], F32, tag="rank_f")
                        nc.vector.tensor_add(out=rank_f[:], in0=rank_ps[:], in1=pad_off[:])
                        nc.vector.tensor_copy(out=rank_i[:, sl], in_=rank_f[:])

                        # group payload scatter (overlaps next group's router)
                        for j in range(GT):
                            i = i0 + j
                            pay = ppool.tile([P, WPAY], F32, tag="pay")
                            eng = nc.scalar if i % 2 == 0 else nc.sync
                            eng.dma_start(out=pay[:, 0:H], in_=h_ext[i * P:(i + 1) * P, :])
                            nc.vector.tensor_copy(out=pay[:, H:H + 1],
                                                  in_=comb_slab[:, i:i + 1])
                            nc.vector.tensor_copy(out=pay[:, H + 1:H + 2],
                                                  in_=tok_slab[:, i:i + 1])
                            nc.gpsimd.indirect_dma_start(
                                out=h_c[:],
                                out_offset=bass.IndirectOffsetOnAxis(
                                    ap=rank_i[:, i:i + 1], axis=0),
                                in_=pay[:], in_offset=None,
                                bounds_check=TCAP - 1, oob_is_err=False)

            # -------- FFN over compact tokens --------
            with tc.tile_pool(name="fpool", bufs=2) as fpool, \
                 tc.tile_pool(name="wpool", bufs=4) as wpool, \
                 tc.tile_pool(name="hcpool", bufs=CT + 2) as hcpool, \
                 tc.tile_pool(name="gpool", bufs=KF) as gpool, \
                 tc.tile_pool(name="w2pool", bufs=KF) as w2pool, \
                 tc.tile_pool(name="opool", bufs=3) as opool, \
                 tc.tile_pool(name="ftrpsum", bufs=1, space="PSUM") as ftrpsum, \
                 tc.tile_pool(name="fpsum", bufs=2, space="PSUM") as fpsum, \
                 tc.tile_pool(name="opsum", bufs=1, space="PSUM") as opsum:

                # w2 resident in bf16
                w2b = []
                for f in range(KF):
                    w2s = fpool.tile([P, H], F32, tag="w2stage")
                    nc.sync.dma_start(out=w2s[:], in_=w2_ext[f * P:(f + 1) * P, :])
                    w2t = w2pool.tile([P, H], BF16, tag="w2b")
                    nc.vector.tensor_copy(out=w2t[:], in_=w2s[:])
                    w2b.append(w2t)

                for c in range(NCH):
                    hcts = []
                    idxs = []
                    for t in range(CT):
                        hct = hcpool.tile([P, WPAY], F32, tag="hc")
                        r0 = c * CH + t * P
                        nc.sync.dma_start(out=hct[:], in_=h_c[r0:r0 + P, :])
                        idx = hcpool.tile([P, 1], I32, tag="idx")
                        nc.vector.tensor_copy(out=idx[:], in_=hct[:, H + 1:H + 2])
                        hcts.append(hct)
                        idxs.append(idx)
                    hTr = fpool.tile([P, KH, CH], F32R, tag="hTr")
                    for t in range(CT):
                        trp = ftrpsum.tile([P, KH, P], F32, tag="ftr")
                        for k in range(KH):
                            nc.tensor.transpose(out=trp[:, k], in_=hcts[t][:, k * P:(k + 1) * P],
                                                identity=ident[:])
                        nc.vector.tensor_copy(out=hTr[:, :, t * P:(t + 1) * P], in_=trp[:])

                    # stage A: G^T tiles [f, tokens]
                    gts = []
                    for f in range(KF):
                        w1s = wpool.tile([P, KH, P], F32R, tag="w1s")
                        nc.sync.dma_start(
                            out=w1s[:],
                            in_=w1_ext[:, f * P:(f + 1) * P].rearrange("(k p) m -> p k m", p=P))
                        w3s = wpool.tile([P, KH, P], F32R, tag="w3s")
                        nc.sync.dma_start(
                            out=w3s[:],
                            in_=w3_ext[:, f * P:(f + 1) * P].rearrange("(k p) m -> p k m", p=P))
                        x1 = fpsum.tile([P, CH], F32, tag="x1")
                        x3 = fpsum.tile([P, CH], F32, tag="x3")
                        for k in range(KH):
                            nc.tensor.matmul(x1[:], lhsT=w1s[:, k], rhs=hTr[:, k],
                                             start=(k == 0), stop=(k == KH - 1))
                        for k in range(KH):
                            nc.tensor.matmul(x3[:], lhsT=w3s[:, k], rhs=hTr[:, k],
                                             start=(k == 0), stop=(k == KH - 1))
                        gate = fpool.tile([P, CH], F32, tag="gate")
                        nc.scalar.activation(gate[:], x1[:], AF.Silu)
                        gt = gpool.tile([P, CH], BF16, tag="G")
                        nc.vector.tensor_mul(out=gt[:], in0=gate[:], in1=x3[:])
                        gts.append(gt)

                    # stage B: out rows, scaled by combine, scattered to scat
                    for t in range(CT):
                        o = opsum.tile([P, H], F32, tag="o")
                        for f in range(KF):
                            for hh in range(NHALF):
                                nc.tensor.matmul(
                                    o[:, hh * HW2:(hh + 1) * HW2],
                                    lhsT=gts[f][:, t * P:(t + 1) * P],
                                    rhs=w2b[f][:, hh * HW2:(hh + 1) * HW2],
                                    start=(f == 0), stop=(f == KF - 1))
                        osb = opool.tile([P, H], BF16, tag="osb")
                        nc.vector.tensor_scalar_mul(osb[:], o[:], hcts[t][:, H:H + 1])
                        nc.gpsimd.indirect_dma_start(
                            out=scat[:],
                            out_offset=bass.IndirectOffsetOnAxis(ap=idxs[t][:, 0:1], axis=0),
                            in_=osb[:], in_offset=None,
                            bounds_check=T + P - 1, oob_is_err=False)

            # -------- collective + output --------
            with tc.tile_pool(name="oc", bufs=2) as ocpool:
                nc.gpsimd.collective_compute(
                    "ReduceScatter", mybir.AluOpType.add,
                    replica_groups=[list(range(n_cores))],
                    ins=[scat[0:T, :]], outs=[rs_out[:]])
                shard = T // n_cores
                for r in range(shard // P):
                    oct_ = ocpool.tile([P, H], BF16, tag="oct")
                    nc.sync.dma_start(out=oct_[:], in_=rs_out[r * P:(r + 1) * P, :])
                    octf = ocpool.tile([P, H], F32, tag="octf")
                    nc.vector.tensor_copy(out=octf[:], in_=oct_[:])
                    nc.sync.dma_start(out=out_ext[r * P:(r + 1) * P, :], in_=octf[:])

    nc.finalize()
    return nc


def kernel(hidden_states, gate_w, w1, w3, w2):
    T, H = hidden_states.shape
    E, _, FF = w1.shape
    n_cores = 8
    nc = build_kernel(T=T, H=H, FF=FF, E=E, n_cores=n_cores)
    onehots = np.eye(E, dtype=np.float32)
    in_maps = []
    for e in range(n_cores):
        in_maps.append({
            "h": np.ascontiguousarray(hidden_states, dtype=np.float32),
            "gate_w": np.ascontiguousarray(gate_w, dtype=np.float32),
            "w1l": np.ascontiguousarray(w1[e], dtype=np.float32),
            "w3l": np.ascontiguousarray(w3[e], dtype=np.float32),
            "w2l": np.ascontiguousarray(w2[e], dtype=np.float32),
            "onehot": np.tile(onehots[e], (128, 1)),
        })
    res = run_bass_kernel_spmd(nc, in_maps, list(range(n_cores))).results
    return np.concatenate([res[i]["out_shard"] for i in range(n_cores)], axis=0)


if __name__ == "__main__":
    nc = build_kernel()
    print("built", len(nc.inst_map), "instructions")


# revision 20
# speedup vs baseline: 1.3822x; 1.1330x over previous
"""Mixtral MoE layer (top-2 of 8 experts) on 8 Trainium2 NeuronCores.

Strategy: expert parallelism. Core e owns expert e's weights (w1/w3/w2[e]).
Each core:
  1. Router (exact fp32): logits = h @ gate_w, top-2 via max8, combine weight
     for own expert via sigmoid(l_e - l_other); builds a compaction rank for
     the tokens routed to this expert (matmul-based prefix sums).
  2. Compaction: payload rows [h | combine | token_id] are indirect-DMA
     scattered into a dense per-expert buffer h_c (capacity TCAP).
  3. FFN over compact tokens (fp32r stage A, bf16 stage B), scaled by the
     combine weight, indirect-scattered to the token's row of a [T,H] buffer.
  4. ReduceScatter(add) across the 8 cores; host concatenates the shards.
"""
import sys

sys.path.insert(0, "/opt/trn_rl_repo")

import numpy as np

import concourse.bass as bass
import concourse.mybir as mybir
from concourse import bacc
from concourse.tile import TileContext
from concourse.tile_rust import add_dep_helper
from concourse.masks import make_identity
from concourse.bass_utils import run_bass_kernel_spmd

F32 = mybir.dt.float32
F32R = mybir.dt.float32r
BF16 = mybir.dt.bfloat16
I32 = mybir.dt.int32
AF = mybir.ActivationFunctionType
P = 128


def build_kernel(T=16384, H=1024, FF=3584, E=8, TCAP=4608, CH=512, n_cores=8):
    NT = T // P      # token tiles
    KH = H // P      # contraction tiles over H
    KF = FF // P     # f tiles (stage A output tiles / stage B contraction)
    NCH = TCAP // CH
    CT = CH // P     # token tiles per FFN chunk
    WPAY = H + 8     # payload row: h | combine | token_id | pad
    TRASH = float(T)  # scatter row for capacity-pad slots
    BIG = 1.0e9
    NHALF = max(1, H // 512)  # stage B free-dim chunks
    HW2 = H // NHALF

    nc = bacc.Bacc(num_devices=n_cores, num_swdge_queues=4)

    h_ext = nc.dram_tensor("h", [T, H], F32, kind="ExternalInput")
    gw_ext = nc.dram_tensor("gate_w", [H, E], F32, kind="ExternalInput")
    w1_ext = nc.dram_tensor("w1l", [H, FF], F32R, kind="ExternalInput")
    w3_ext = nc.dram_tensor("w3l", [H, FF], F32R, kind="ExternalInput")
    w2_ext = nc.dram_tensor("w2l", [FF, H], F32, kind="ExternalInput")
    oh_ext = nc.dram_tensor("onehot", [P, E], F32, kind="ExternalInput")
    out_ext = nc.dram_tensor("out_shard", [T // n_cores, H], F32, kind="ExternalOutput")

    h_c = nc.dram_tensor("h_c", [TCAP, WPAY], F32)
    scat = nc.dram_tensor("scat", [T + P, H], BF16)
    rs_out = nc.dram_tensor("rs_out", [T // n_cores, H], BF16)

    tok_ids = np.arange(T, dtype=np.float32).reshape(NT, P).T.copy()  # [P, NT]
    tok_const = nc.inline_tensor(tok_ids, name="tok_ids")
    ustrict_np = np.triu(np.ones((P, P), dtype=np.float32), 1)  # [k, m] = 1 iff k < m
    ustrict_const = nc.inline_tensor(ustrict_np, name="ustrict")

    with TileContext(nc) as tc:
        with tc.tile_pool(name="const", bufs=1) as cpool:
            ident = cpool.tile([P, P], F32)
            make_identity(nc, ident[:])
            ustrict = cpool.tile([P, P], F32)
            nc.sync.dma_start(out=ustrict[:], in_=ustrict_const[:])
            tok_slab = cpool.tile([P, NT], F32)
            nc.sync.dma_start(out=tok_slab[:], in_=tok_const[:])
            ones_col = cpool.tile([P, 1], F32)
            nc.vector.memset(ones_col[:], 1.0)
            ones_row = cpool.tile([1, P], F32)
            nc.vector.memset(ones_row[:], 1.0)
            gw_sb = cpool.tile([P, KH, E], F32)
            nc.sync.dma_start(out=gw_sb[:], in_=gw_ext[:].rearrange("(k p) e -> p k e", p=P))
            oh_sb = cpool.tile([P, E], F32)
            nc.sync.dma_start(out=oh_sb[:], in_=oh_ext[:])
            zrow = cpool.tile([P, WPAY], F32)
            nc.vector.memset(zrow[:], 0.0)
            nc.vector.memset(zrow[:, H + 1:H + 2], TRASH)
            zrow_b = cpool.tile([P, H], BF16)
            nc.vector.memset(zrow_b[:], 0.0)
            zer_row = cpool.tile([1, P], F32)
            nc.vector.memset(zer_row[:], 0.0)

            # -------- router + compaction + payload, in overlapped groups --------
            # Tokens are processed in NG groups of GT tiles. Each group computes
            # its logits/top2/combine, then its compaction ranks; the global rank
            # base is carried between groups by chaining tensor_tensor_scan
            # (initial = previous group's last inclusive prefix). A group's
            # payload scatter only depends on its own ranks, so it overlaps the
            # next group's router compute/DMA instead of serializing at the end.
            # -------- zero-fill h_c and scat (batched, SWDGE queues) --------
            ZB = 4
            for r in range(TCAP // (P * ZB)):
                nc.gpsimd.dma_start(
                    out=h_c[r * P * ZB:(r + 1) * P * ZB, :].rearrange(
                        "(a p) w -> p a w", p=P),
                    in_=zrow[:, None, :].to_broadcast([P, ZB, WPAY]))
            NSC = (T + P) // P
            for r in range(NSC // ZB):
                nc.gpsimd.dma_start(
                    out=scat[r * P * ZB:(r + 1) * P * ZB, :].rearrange(
                        "(a p) w -> p a w", p=P),
                    in_=zrow_b[:, None, :].to_broadcast([P, ZB, H]))
            for r in range((NSC // ZB) * ZB, NSC):
                nc.gpsimd.dma_start(out=scat[r * P:(r + 1) * P, :], in_=zrow_b[:])

            GT = min(16, NT)
            NG = NT // GT
            scatter_insts = []
            with tc.tile_pool(name="rslab", bufs=1) as spool:
                mx_slab = spool.tile([P, NT, 8], F32)
                comb_slab = spool.tile([P, NT], F32)
                rank_i = spool.tile([P, NT], I32)
                cs_slab = spool.tile([1, NT], F32)
                incl_slab = spool.tile([1, NT], F32)

                with tc.tile_pool(name="rtile", bufs=3) as rpool, \
                     tc.tile_pool(name="htpool", bufs=GT + 2) as htpool, \
                     tc.tile_pool(name="rgrp", bufs=2) as gpool_r, \
                     tc.tile_pool(name="rpsum", bufs=2, space="PSUM") as rpsum, \
                     tc.tile_pool(name="rcpsum", bufs=1, space="PSUM") as rcpsum:
                    for q in range(NG):
                        i0 = q * GT
                        lg_g = gpool_r.tile([P, GT, E], F32, tag="lg_g")
                        hts = []
                        for j in range(GT):
                            i = i0 + j
                            ht = htpool.tile([P, WPAY], F32, tag="ht")
                            hts.append(ht)
                            heng = nc.sync if i % 2 == 0 else nc.scalar
                            heng.dma_start(out=ht[:, 0:H], in_=h_ext[i * P:(i + 1) * P, :])
                            trp = rpsum.tile([P, KH, P], F32, tag="trp")
                            for k in range(KH):
                                nc.tensor.transpose(out=trp[:, k],
                                                    in_=ht[:, k * P:(k + 1) * P],
                                                    identity=ident[:])
                            hTt = rpool.tile([P, KH, P], F32, tag="hT")
                            if i % 2 == 0:
                                nc.vector.tensor_copy(out=hTt[:], in_=trp[:])
                            else:
                                nc.scalar.copy(out=hTt[:], in_=trp[:])
                            lg = rpsum.tile([P, E], F32, tag="lg")
                            for k in range(KH):
                                nc.tensor.matmul(lg[:], lhsT=hTt[:, k], rhs=gw_sb[:, k],
                                                 start=(k == 0), stop=(k == KH - 1))
                            nc.scalar.copy(out=lg_g[:, j], in_=lg[:])
                            nc.vector.max(out=mx_slab[:, i], in_=lg_g[:, j])

                        # group combine/mask
                        sl = slice(i0, i0 + GT)
                        tmp_le = gpool_r.tile([P, GT, E], F32, tag="tmp_le")
                        nc.vector.tensor_mul(out=tmp_le[:], in0=lg_g[:],
                                             in1=oh_sb[:, None, :].to_broadcast([P, GT, E]))
                        le = gpool_r.tile([P, GT], F32, tag="le")
                        nc.vector.tensor_reduce(out=le[:], in_=tmp_le[:],
                                                axis=mybir.AxisListType.X,
                                                op=mybir.AluOpType.add)
                        m1 = mx_slab[:, sl, 0]
                        m2 = mx_slab[:, sl, 1]
                        msum = gpool_r.tile([P, GT], F32, tag="msum")
                        nc.vector.tensor_add(out=msum[:], in0=m1, in1=m2)
                        sgin = gpool_r.tile([P, GT], F32, tag="sgin")
                        nc.vector.tensor_scalar_mul(sgin[:], le[:], 2.0)
                        nc.vector.tensor_sub(out=sgin[:], in0=sgin[:], in1=msum[:])
                        sig = gpool_r.tile([P, GT], F32, tag="sig")
                        nc.scalar.activation(sig[:], sgin[:], AF.Sigmoid)
                        eq1 = gpool_r.tile([P, GT], F32, tag="eq1")
                        eq2 = gpool_r.tile([P, GT], F32, tag="eq2")
                        nc.vector.tensor_tensor(out=eq1[:], in0=le[:], in1=m1,
                                                op=mybir.AluOpType.is_equal)
                        nc.vector.tensor_tensor(out=eq2[:], in0=le[:], in1=m2,
                                                op=mybir.AluOpType.is_equal)
                        mask_g = gpool_r.tile([P, GT], F32, tag="mask_g")
                        nc.vector.tensor_add(out=mask_g[:], in0=eq1[:], in1=eq2[:])
                        nc.vector.tensor_mul(out=comb_slab[:, sl], in0=mask_g[:], in1=sig[:])

                        # group compaction ranks with chained global base
                        csum_ps = rcpsum.tile([1, GT], F32, tag="c1")
                        nc.tensor.matmul(csum_ps[:], lhsT=ones_col[:], rhs=mask_g[:],
                                         start=True, stop=True)
                        nc.vector.tensor_copy(out=cs_slab[:, sl], in_=csum_ps[:])
                        init = 0.0 if q == 0 else incl_slab[:, i0 - 1:i0]
                        nc.vector.tensor_tensor_scan(out=incl_slab[:, sl],
                                                     data0=cs_slab[:, sl],
                                                     data1=zer_row[:, 0:GT],
                                                     initial=init,
                                                     op0=mybir.AluOpType.add,
                                                     op1=mybir.AluOpType.add)
                        cpref = gpool_r.tile([1, GT], F32, tag="cpref")
                        nc.vector.tensor_sub(out=cpref[:], in0=incl_slab[:, sl],
                                             in1=cs_slab[:, sl])
                        rank_ps = rcpsum.tile([P, GT], F32, tag="rk")
                        nc.tensor.matmul(rank_ps[:], lhsT=ustrict[:], rhs=mask_g[:],
                                         start=True, stop=False)
                        nc.tensor.matmul(rank_ps[:], lhsT=ones_row[:], rhs=cpref[:],
                                         start=False, stop=True)
                        pad_off = gpool_r.tile([P, GT], F32, tag="pad_off")
                        nc.vector.tensor_scalar(out=pad_off[:], in0=mask_g[:],
                                                scalar1=-BIG, scalar2=BIG,
                                                op0=mybir.AluOpType.mult,
                                                op1=mybir.AluOpType.add)
                        rank_f = gpool_r.tile([P, GT], F32, tag="rank_f")
                        nc.vector.tensor_add(out=rank_f[:], in0=rank_ps[:], in1=pad_off[:])
                        nc.vector.tensor_copy(out=rank_i[:, sl], in_=rank_f[:])

                        # group payload scatter (overlaps next group's router)
                        for j in range(GT):
                            i = i0 + j
                            pay = hts[j]
                            nc.vector.tensor_copy(out=pay[:, H:H + 1],
                                                  in_=comb_slab[:, i:i + 1])
                            nc.vector.tensor_copy(out=pay[:, H + 1:H + 2],
                                                  in_=tok_slab[:, i:i + 1])
                            claim = bass.AP(
                                tensor=h_c[0:P, :].tensor, offset=0,
                                ap=h_c[0:P, :].ap,
                                dep_tracking_offset=(i % (TCAP // P)) * P * WPAY)
                            sc = nc.gpsimd.indirect_dma_start(
                                out=claim,
                                out_offset=bass.IndirectOffsetOnAxis(
                                    ap=rank_i[:, i:i + 1], axis=0),
                                in_=pay[:], in_offset=None,
                                bounds_check=TCAP - 1, oob_is_err=False)
                            sc.ins.queue = "qPoolDynamic" + str(i % 4 or '')
                            scatter_insts.append(sc.ins)

            # fence: all payload scatters complete before any h_c chunk read
            fence = nc.gpsimd.nop(hint="hc_fence", nofuse=True)
            for si in scatter_insts:
                add_dep_helper(fence.ins, si, True, "hc scatter fence")

            # -------- FFN over compact tokens --------
            with tc.tile_pool(name="fpool", bufs=2) as fpool, \
                 tc.tile_pool(name="wpool", bufs=4) as wpool, \
                 tc.tile_pool(name="hcpool", bufs=CT + 2) as hcpool, \
                 tc.tile_pool(name="gpool", bufs=KF) as gpool, \
                 tc.tile_pool(name="w2pool", bufs=KF) as w2pool, \
                 tc.tile_pool(name="opool", bufs=3) as opool, \
                 tc.tile_pool(name="ftrpsum", bufs=1, space="PSUM") as ftrpsum, \
                 tc.tile_pool(name="fpsum", bufs=2, space="PSUM") as fpsum, \
                 tc.tile_pool(name="opsum", bufs=1, space="PSUM") as opsum:

                # w2 resident in bf16
                w2b = []
                for f in range(KF):
                    w2s = fpool.tile([P, H], F32, tag="w2stage")
                    nc.sync.dma_start(out=w2s[:], in_=w2_ext[f * P:(f + 1) * P, :])
                    w2t = w2pool.tile([P, H], BF16, tag="w2b")
                    nc.vector.tensor_copy(out=w2t[:], in_=w2s[:])
                    w2b.append(w2t)

                for c in range(NCH):
                    hcts = []
                    idxs = []
                    for t in range(CT):
                        hct = hcpool.tile([P, WPAY], F32, tag="hc")
                        r0 = c * CH + t * P
                        ld = nc.sync.dma_start(out=hct[:], in_=h_c[r0:r0 + P, :])
                        add_dep_helper(ld.ins, fence.ins, True, "hc fence")
                        idx = hcpool.tile([P, 1], I32, tag="idx")
                        nc.vector.tensor_copy(out=idx[:], in_=hct[:, H + 1:H + 2])
                        hcts.append(hct)
                        idxs.append(idx)
                    hTr = fpool.tile([P, KH, CH], F32R, tag="hTr")
                    for t in range(CT):
                        trp = ftrpsum.tile([P, KH, P], F32, tag="ftr")
                        for k in range(KH):
                            nc.tensor.transpose(out=trp[:, k], in_=hcts[t][:, k * P:(k + 1) * P],
                                                identity=ident[:])
                        nc.vector.tensor_copy(out=hTr[:, :, t * P:(t + 1) * P], in_=trp[:])

                    # stage A: G^T tiles [f, tokens]
                    gts = []
                    for f in range(KF):
                        w1s = wpool.tile([P, KH, P], F32R, tag="w1s")
                        nc.sync.dma_start(
                            out=w1s[:],
                            in_=w1_ext[:, f * P:(f + 1) * P].rearrange("(k p) m -> p k m", p=P))
                        w3s = wpool.tile([P, KH, P], F32R, tag="w3s")
                        nc.sync.dma_start(
                            out=w3s[:],
                            in_=w3_ext[:, f * P:(f + 1) * P].rearrange("(k p) m -> p k m", p=P))
                        x1 = fpsum.tile([P, CH], F32, tag="x1")
                        x3 = fpsum.tile([P, CH], F32, tag="x3")
                        for k in range(KH):
                            nc.tensor.matmul(x1[:], lhsT=w1s[:, k], rhs=hTr[:, k],
                                             start=(k == 0), stop=(k == KH - 1))
                        for k in range(KH):
                            nc.tensor.matmul(x3[:], lhsT=w3s[:, k], rhs=hTr[:, k],
                                             start=(k == 0), stop=(k == KH - 1))
                        gate = fpool.tile([P, CH], F32, tag="gate")
                        nc.scalar.activation(gate[:], x1[:], AF.Silu)
                        gt = gpool.tile([P, CH], BF16, tag="G")
                        nc.vector.tensor_mul(out=gt[:], in0=gate[:], in1=x3[:])
                        gts.append(gt)

                    # stage B: out rows, scaled by combine, scattered to scat
                    for t in range(CT):
                        o = opsum.tile([P, H], F32, tag="o")
                        for f in range(KF):
                            for hh in range(NHALF):
                                nc.tensor.matmul(
                                    o[:, hh * HW2:(hh + 1) * HW2],
                                    lhsT=gts[f][:, t * P:(t + 1) * P],
                                    rhs=w2b[f][:, hh * HW2:(hh + 1) * HW2],
                                    start=(f == 0), stop=(f == KF - 1))
                        osb = opool.tile([P, H], BF16, tag="osb")
                        nc.vector.tensor_scalar_mul(osb[:], o[:], hcts[t][:, H:H + 1])
                        oclaim = bass.AP(
                            tensor=scat[0:P, :].tensor, offset=0,
                            ap=scat[0:P, :].ap,
                            dep_tracking_offset=(c * CT + t) * P * H)
                        sco = nc.gpsimd.indirect_dma_start(
                            out=oclaim,
                            out_offset=bass.IndirectOffsetOnAxis(ap=idxs[t][:, 0:1], axis=0),
                            in_=osb[:], in_offset=None,
                            bounds_check=T + P - 1, oob_is_err=False)
                        sco.ins.queue = "qPoolDynamic" + str((c * CT + t) % 4 or '')

            # -------- collective + output --------
            with tc.tile_pool(name="oc", bufs=2) as ocpool:
                nc.gpsimd.collective_compute(
                    "ReduceScatter", mybir.AluOpType.add,
                    replica_groups=[list(range(n_cores))],
                    ins=[scat[0:T, :]], outs=[rs_out[:]])
                shard = T // n_cores
                for r in range(shard // P):
                    oct_ = ocpool.tile([P, H], BF16, tag="oct")
                    nc.sync.dma_start(out=oct_[:], in_=rs_out[r * P:(r + 1) * P, :])
                    octf = ocpool.tile([P, H], F32, tag="octf")
                    nc.vector.tensor_copy(out=octf[:], in_=oct_[:])
                    nc.sync.dma_start(out=out_ext[r * P:(r + 1) * P, :], in_=octf[:])

    nc.finalize()
    return nc


def kernel(hidden_states, gate_w, w1, w3, w2):
    T, H = hidden_states.shape
    E, _, FF = w1.shape
    n_cores = 8
    nc = build_kernel(T=T, H=H, FF=FF, E=E, n_cores=n_cores)
    onehots = np.eye(E, dtype=np.float32)
    in_maps = []
    for e in range(n_cores):
        in_maps.append({
            "h": np.ascontiguousarray(hidden_states, dtype=np.float32),
            "gate_w": np.ascontiguousarray(gate_w, dtype=np.float32),
            "w1l": np.ascontiguousarray(w1[e], dtype=np.float32),
            "w3l": np.ascontiguousarray(w3[e], dtype=np.float32),
            "w2l": np.ascontiguousarray(w2[e], dtype=np.float32),
            "onehot": np.tile(onehots[e], (128, 1)),
        })
    res = run_bass_kernel_spmd(nc, in_maps, list(range(n_cores))).results
    return np.concatenate([res[i]["out_shard"] for i in range(n_cores)], axis=0)


if __name__ == "__main__":
    nc = build_kernel()
    print("built", len(nc.inst_map), "instructions")


# revision 23
# speedup vs baseline: 1.3880x; 1.0042x over previous
"""Mixtral MoE layer (top-2 of 8 experts) on 8 Trainium2 NeuronCores.

Strategy: expert parallelism. Core e owns expert e's weights (w1/w3/w2[e]).
Each core:
  1. Router (exact fp32): logits = h @ gate_w, top-2 via max8, combine weight
     for own expert via sigmoid(l_e - l_other); builds a compaction rank for
     the tokens routed to this expert (matmul-based prefix sums).
  2. Compaction: payload rows [h | combine | token_id] are indirect-DMA
     scattered into a dense per-expert buffer h_c (capacity TCAP).
  3. FFN over compact tokens (fp32r stage A, bf16 stage B), scaled by the
     combine weight, indirect-scattered to the token's row of a [T,H] buffer.
  4. ReduceScatter(add) across the 8 cores; host concatenates the shards.
"""
import sys

sys.path.insert(0, "/opt/trn_rl_repo")

import numpy as np

import concourse.bass as bass
import concourse.mybir as mybir
from concourse import bacc
from concourse.tile import TileContext
from concourse.tile_rust import add_dep_helper
from concourse.masks import make_identity
from concourse.bass_utils import run_bass_kernel_spmd

F32 = mybir.dt.float32
F32R = mybir.dt.float32r
BF16 = mybir.dt.bfloat16
I32 = mybir.dt.int32
AF = mybir.ActivationFunctionType
P = 128


def build_kernel(T=16384, H=1024, FF=3584, E=8, TCAP=4608, CH=512, n_cores=8):
    NT = T // P      # token tiles
    KH = H // P      # contraction tiles over H
    KF = FF // P     # f tiles (stage A output tiles / stage B contraction)
    NCH = TCAP // CH
    CT = CH // P     # token tiles per FFN chunk
    WPAY = H + 8     # payload row: h | combine | token_id | pad
    TRASH = float(T)  # scatter row for capacity-pad slots
    BIG = 1.0e9
    NHALF = max(1, H // 512)  # stage B free-dim chunks
    HW2 = H // NHALF

    nc = bacc.Bacc(num_devices=n_cores, num_swdge_queues=4)

    h_ext = nc.dram_tensor("h", [T, H], F32, kind="ExternalInput")
    gw_ext = nc.dram_tensor("gate_w", [H, E], F32, kind="ExternalInput")
    w1_ext = nc.dram_tensor("w1l", [H, FF], F32R, kind="ExternalInput")
    w3_ext = nc.dram_tensor("w3l", [H, FF], F32R, kind="ExternalInput")
    w2_ext = nc.dram_tensor("w2l", [FF, H], F32, kind="ExternalInput")
    oh_ext = nc.dram_tensor("onehot", [P, E], F32, kind="ExternalInput")
    out_ext = nc.dram_tensor("out_shard", [T // n_cores, H], F32, kind="ExternalOutput")

    h_c = nc.dram_tensor("h_c", [TCAP, WPAY], F32)
    scat = nc.dram_tensor("scat", [T + P, H], BF16)
    rs_out = nc.dram_tensor("rs_out", [T // n_cores, H], BF16)

    tok_ids = np.arange(T, dtype=np.float32).reshape(NT, P).T.copy()  # [P, NT]
    tok_const = nc.inline_tensor(tok_ids, name="tok_ids")
    ustrict_np = np.triu(np.ones((P, P), dtype=np.float32), 1)  # [k, m] = 1 iff k < m
    ustrict_const = nc.inline_tensor(ustrict_np, name="ustrict")

    with TileContext(nc) as tc:
        with tc.tile_pool(name="const", bufs=1) as cpool:
            ident = cpool.tile([P, P], F32)
            make_identity(nc, ident[:])
            ustrict = cpool.tile([P, P], F32)
            nc.sync.dma_start(out=ustrict[:], in_=ustrict_const[:])
            tok_slab = cpool.tile([P, NT], F32)
            nc.sync.dma_start(out=tok_slab[:], in_=tok_const[:])
            ones_col = cpool.tile([P, 1], F32)
            nc.vector.memset(ones_col[:], 1.0)
            ones_row = cpool.tile([1, P], F32)
            nc.vector.memset(ones_row[:], 1.0)
            gw_sb = cpool.tile([P, KH, E], F32)
            nc.sync.dma_start(out=gw_sb[:], in_=gw_ext[:].rearrange("(k p) e -> p k e", p=P))
            oh_sb = cpool.tile([P, E], F32)
            nc.sync.dma_start(out=oh_sb[:], in_=oh_ext[:])
            zrow = cpool.tile([P, WPAY], F32)
            nc.vector.memset(zrow[:], 0.0)
            nc.vector.memset(zrow[:, H + 1:H + 2], TRASH)
            zrow_b = cpool.tile([P, H], BF16)
            nc.vector.memset(zrow_b[:], 0.0)
            zer_row = cpool.tile([1, P], F32)
            nc.vector.memset(zer_row[:], 0.0)

            # -------- router + compaction + payload, in overlapped groups --------
            # Tokens are processed in NG groups of GT tiles. Each group computes
            # its logits/top2/combine, then its compaction ranks; the global rank
            # base is carried between groups by chaining tensor_tensor_scan
            # (initial = previous group's last inclusive prefix). A group's
            # payload scatter only depends on its own ranks, so it overlaps the
            # next group's router compute/DMA instead of serializing at the end.
            # -------- zero-fill h_c and scat (batched, SWDGE queues) --------
            ZB = 4
            for r in range(TCAP // (P * ZB)):
                nc.gpsimd.dma_start(
                    out=h_c[r * P * ZB:(r + 1) * P * ZB, :].rearrange(
                        "(a p) w -> p a w", p=P),
                    in_=zrow[:, None, :].to_broadcast([P, ZB, WPAY]))
            NSC = (T + P) // P
            for r in range(NSC // ZB):
                nc.gpsimd.dma_start(
                    out=scat[r * P * ZB:(r + 1) * P * ZB, :].rearrange(
                        "(a p) w -> p a w", p=P),
                    in_=zrow_b[:, None, :].to_broadcast([P, ZB, H]))
            for r in range((NSC // ZB) * ZB, NSC):
                nc.gpsimd.dma_start(out=scat[r * P:(r + 1) * P, :], in_=zrow_b[:])

            GT = min(16, NT)
            NG = NT // GT
            scatter_insts = []
            with tc.tile_pool(name="rslab", bufs=1) as spool:
                mx_slab = spool.tile([P, NT, 8], F32)
                comb_slab = spool.tile([P, NT], F32)
                rank_i = spool.tile([P, NT], I32)
                cs_slab = spool.tile([1, NT], F32)
                incl_slab = spool.tile([1, NT], F32)

                with tc.tile_pool(name="rtile", bufs=3) as rpool, \
                     tc.tile_pool(name="rgrp", bufs=2) as gpool_r, \
                     tc.tile_pool(name="ppool", bufs=8) as ppool, \
                     tc.tile_pool(name="rpsum", bufs=2, space="PSUM") as rpsum, \
                     tc.tile_pool(name="rcpsum", bufs=1, space="PSUM") as rcpsum:
                    for q in range(NG):
                        i0 = q * GT
                        lg_g = gpool_r.tile([P, GT, E], F32, tag="lg_g")
                        for j in range(GT):
                            i = i0 + j
                            ht = rpool.tile([P, H], F32, tag="ht")
                            heng = nc.sync if i % 2 == 0 else nc.scalar
                            heng.dma_start(out=ht[:], in_=h_ext[i * P:(i + 1) * P, :])
                            trp = rpsum.tile([P, KH, P], F32, tag="trp")
                            for k in range(KH):
                                nc.tensor.transpose(out=trp[:, k],
                                                    in_=ht[:, k * P:(k + 1) * P],
                                                    identity=ident[:])
                            hTt = rpool.tile([P, KH, P], F32, tag="hT")
                            if i % 2 == 0:
                                nc.vector.tensor_copy(out=hTt[:], in_=trp[:])
                            else:
                                nc.scalar.copy(out=hTt[:], in_=trp[:])
                            lg = rpsum.tile([P, E], F32, tag="lg")
                            for k in range(KH):
                                nc.tensor.matmul(lg[:], lhsT=hTt[:, k], rhs=gw_sb[:, k],
                                                 start=(k == 0), stop=(k == KH - 1))
                            nc.scalar.copy(out=lg_g[:, j], in_=lg[:])
                            nc.vector.max(out=mx_slab[:, i], in_=lg_g[:, j])

                        # group combine/mask
                        sl = slice(i0, i0 + GT)
                        tmp_le = gpool_r.tile([P, GT, E], F32, tag="tmp_le")
                        nc.vector.tensor_mul(out=tmp_le[:], in0=lg_g[:],
                                             in1=oh_sb[:, None, :].to_broadcast([P, GT, E]))
                        le = gpool_r.tile([P, GT], F32, tag="le")
                        nc.vector.tensor_reduce(out=le[:], in_=tmp_le[:],
                                                axis=mybir.AxisListType.X,
                                                op=mybir.AluOpType.add)
                        m1 = mx_slab[:, sl, 0]
                        m2 = mx_slab[:, sl, 1]
                        msum = gpool_r.tile([P, GT], F32, tag="msum")
                        nc.vector.tensor_add(out=msum[:], in0=m1, in1=m2)
                        sgin = gpool_r.tile([P, GT], F32, tag="sgin")
                        nc.vector.tensor_scalar_mul(sgin[:], le[:], 2.0)
                        nc.vector.tensor_sub(out=sgin[:], in0=sgin[:], in1=msum[:])
                        sig = gpool_r.tile([P, GT], F32, tag="sig")
                        nc.scalar.activation(sig[:], sgin[:], AF.Sigmoid)
                        eq1 = gpool_r.tile([P, GT], F32, tag="eq1")
                        eq2 = gpool_r.tile([P, GT], F32, tag="eq2")
                        nc.vector.tensor_tensor(out=eq1[:], in0=le[:], in1=m1,
                                                op=mybir.AluOpType.is_equal)
                        nc.vector.tensor_tensor(out=eq2[:], in0=le[:], in1=m2,
                                                op=mybir.AluOpType.is_equal)
                        mask_g = gpool_r.tile([P, GT], F32, tag="mask_g")
                        nc.vector.tensor_add(out=mask_g[:], in0=eq1[:], in1=eq2[:])
                        nc.vector.tensor_mul(out=comb_slab[:, sl], in0=mask_g[:], in1=sig[:])

                        # group compaction ranks with chained global base
                        csum_ps = rcpsum.tile([1, GT], F32, tag="c1")
                        nc.tensor.matmul(csum_ps[:], lhsT=ones_col[:], rhs=mask_g[:],
                                         start=True, stop=True)
                        nc.vector.tensor_copy(out=cs_slab[:, sl], in_=csum_ps[:])
                        init = 0.0 if q == 0 else incl_slab[:, i0 - 1:i0]
                        nc.vector.tensor_tensor_scan(out=incl_slab[:, sl],
                                                     data0=cs_slab[:, sl],
                                                     data1=zer_row[:, 0:GT],
                                                     initial=init,
                                                     op0=mybir.AluOpType.add,
                                                     op1=mybir.AluOpType.add)
                        cpref = gpool_r.tile([1, GT], F32, tag="cpref")
                        nc.vector.tensor_sub(out=cpref[:], in0=incl_slab[:, sl],
                                             in1=cs_slab[:, sl])
                        rank_ps = rcpsum.tile([P, GT], F32, tag="rk")
                        nc.tensor.matmul(rank_ps[:], lhsT=ustrict[:], rhs=mask_g[:],
                                         start=True, stop=False)
                        nc.tensor.matmul(rank_ps[:], lhsT=ones_row[:], rhs=cpref[:],
                                         start=False, stop=True)
                        pad_off = gpool_r.tile([P, GT], F32, tag="pad_off")
                        nc.vector.tensor_scalar(out=pad_off[:], in0=mask_g[:],
                                                scalar1=-BIG, scalar2=BIG,
                                                op0=mybir.AluOpType.mult,
                                                op1=mybir.AluOpType.add)
                        rank_f = gpool_r.tile([P, GT], F32, tag="rank_f")
                        nc.vector.tensor_add(out=rank_f[:], in0=rank_ps[:], in1=pad_off[:])
                        nc.vector.tensor_copy(out=rank_i[:, sl], in_=rank_f[:])

                        # group payload scatter (overlaps next group's router)
                        for j in range(GT):
                            i = i0 + j
                            pay = ppool.tile([P, WPAY], F32, tag="pay")
                            eng = nc.scalar if i % 2 == 0 else nc.sync
                            eng.dma_start(out=pay[:, 0:H], in_=h_ext[i * P:(i + 1) * P, :])
                            nc.vector.tensor_copy(out=pay[:, H:H + 1],
                                                  in_=comb_slab[:, i:i + 1])
                            nc.vector.tensor_copy(out=pay[:, H + 1:H + 2],
                                                  in_=tok_slab[:, i:i + 1])
                            claim = bass.AP(
                                tensor=h_c[0:P, :].tensor, offset=0,
                                ap=h_c[0:P, :].ap,
                                dep_tracking_offset=(i % (TCAP // P)) * P * WPAY)
                            sc = nc.gpsimd.indirect_dma_start(
                                out=claim,
                                out_offset=bass.IndirectOffsetOnAxis(
                                    ap=rank_i[:, i:i + 1], axis=0),
                                in_=pay[:], in_offset=None,
                                bounds_check=TCAP - 1, oob_is_err=False)
                            sc.ins.queue = "qPoolDynamic" + str(i % 4 or '')
                            scatter_insts.append(sc.ins)

            # fence: all payload scatters complete before any h_c chunk read
            fence = nc.gpsimd.nop(hint="hc_fence", nofuse=True)
            for si in scatter_insts:
                add_dep_helper(fence.ins, si, True, "hc scatter fence")

            # -------- FFN over compact tokens --------
            with tc.tile_pool(name="fpool", bufs=2) as fpool, \
                 tc.tile_pool(name="wpool", bufs=4) as wpool, \
                 tc.tile_pool(name="hcpool", bufs=CT + 2) as hcpool, \
                 tc.tile_pool(name="gpool", bufs=KF) as gpool, \
                 tc.tile_pool(name="w2pool", bufs=KF) as w2pool, \
                 tc.tile_pool(name="opool", bufs=3) as opool, \
                 tc.tile_pool(name="ftrpsum", bufs=1, space="PSUM") as ftrpsum, \
                 tc.tile_pool(name="fpsum", bufs=2, space="PSUM") as fpsum, \
                 tc.tile_pool(name="opsum", bufs=1, space="PSUM") as opsum:

                # w2 resident in bf16
                w2b = []
                for f in range(KF):
                    w2s = fpool.tile([P, H], F32, tag="w2stage")
                    nc.sync.dma_start(out=w2s[:], in_=w2_ext[f * P:(f + 1) * P, :])
                    w2t = w2pool.tile([P, H], BF16, tag="w2b")
                    nc.vector.tensor_copy(out=w2t[:], in_=w2s[:])
                    w2b.append(w2t)

                for c in range(NCH):
                    hcts = []
                    idxs = []
                    for t in range(CT):
                        hct = hcpool.tile([P, WPAY], F32, tag="hc")
                        r0 = c * CH + t * P
                        ld = nc.sync.dma_start(out=hct[:], in_=h_c[r0:r0 + P, :])
                        add_dep_helper(ld.ins, fence.ins, True, "hc fence")
                        idx = hcpool.tile([P, 1], I32, tag="idx")
                        nc.vector.tensor_copy(out=idx[:], in_=hct[:, H + 1:H + 2])
                        hcts.append(hct)
                        idxs.append(idx)
                    hTr = fpool.tile([P, KH, CH], F32R, tag="hTr")
                    for t in range(CT):
                        trp = ftrpsum.tile([P, KH, P], F32, tag="ftr")
                        for k in range(KH):
                            nc.tensor.transpose(out=trp[:, k], in_=hcts[t][:, k * P:(k + 1) * P],
                                                identity=ident[:])
                        nc.vector.tensor_copy(out=hTr[:, :, t * P:(t + 1) * P], in_=trp[:])

                    # stage A: G^T tiles [f, tokens]
                    gts = []
                    for f in range(KF):
                        w1s = wpool.tile([P, KH, P], F32R, tag="w1s")
                        nc.sync.dma_start(
                            out=w1s[:],
                            in_=w1_ext[:, f * P:(f + 1) * P].rearrange("(k p) m -> p k m", p=P))
                        w3s = wpool.tile([P, KH, P], F32R, tag="w3s")
                        nc.sync.dma_start(
                            out=w3s[:],
                            in_=w3_ext[:, f * P:(f + 1) * P].rearrange("(k p) m -> p k m", p=P))
                        x1 = fpsum.tile([P, CH], F32, tag="x1")
                        x3 = fpsum.tile([P, CH], F32, tag="x3")
                        for k in range(KH):
                            nc.tensor.matmul(x1[:], lhsT=w1s[:, k], rhs=hTr[:, k],
                                             start=(k == 0), stop=(k == KH - 1))
                        for k in range(KH):
                            nc.tensor.matmul(x3[:], lhsT=w3s[:, k], rhs=hTr[:, k],
                                             start=(k == 0), stop=(k == KH - 1))
                        gate = fpool.tile([P, CH], F32, tag="gate")
                        nc.scalar.activation(gate[:], x1[:], AF.Silu)
                        gt = gpool.tile([P, CH], BF16, tag="G")
                        nc.vector.tensor_mul(out=gt[:], in0=gate[:], in1=x3[:])
                        gts.append(gt)

                    # stage B: out rows, scaled by combine, scattered to scat
                    for t in range(CT):
                        o = opsum.tile([P, H], F32, tag="o")
                        for f in range(KF):
                            for hh in range(NHALF):
                                nc.tensor.matmul(
                                    o[:, hh * HW2:(hh + 1) * HW2],
                                    lhsT=gts[f][:, t * P:(t + 1) * P],
                                    rhs=w2b[f][:, hh * HW2:(hh + 1) * HW2],
                                    start=(f == 0), stop=(f == KF - 1))
                        osb = opool.tile([P, H], BF16, tag="osb")
                        nc.vector.tensor_scalar_mul(osb[:], o[:], hcts[t][:, H:H + 1])
                        oclaim = bass.AP(
                            tensor=scat[0:P, :].tensor, offset=0,
                            ap=scat[0:P, :].ap,
                            dep_tracking_offset=(c * CT + t) * P * H)
                        sco = nc.gpsimd.indirect_dma_start(
                            out=oclaim,
                            out_offset=bass.IndirectOffsetOnAxis(ap=idxs[t][:, 0:1], axis=0),
                            in_=osb[:], in_offset=None,
                            bounds_check=T + P - 1, oob_is_err=False)
                        sco.ins.queue = "qPoolDynamic" + str((c * CT + t) % 4 or '')

            # -------- collective + output --------
            with tc.tile_pool(name="oc", bufs=2) as ocpool:
                nc.gpsimd.collective_compute(
                    "ReduceScatter", mybir.AluOpType.add,
                    replica_groups=[list(range(n_cores))],
                    ins=[scat[0:T, :]], outs=[rs_out[:]])
                shard = T // n_cores
                for r in range(shard // P):
                    oct_ = ocpool.tile([P, H], BF16, tag="oct")
                    nc.sync.dma_start(out=oct_[:], in_=rs_out[r * P:(r + 1) * P, :])
                    octf = ocpool.tile([P, H], F32, tag="octf")
                    nc.vector.tensor_copy(out=octf[:], in_=oct_[:])
                    nc.sync.dma_start(out=out_ext[r * P:(r + 1) * P, :], in_=octf[:])

    nc.finalize()
    return nc


def kernel(hidden_states, gate_w, w1, w3, w2):
    T, H = hidden_states.shape
    E, _, FF = w1.shape
    n_cores = 8
    nc = build_kernel(T=T, H=H, FF=FF, E=E, n_cores=n_cores)
    onehots = np.eye(E, dtype=np.float32)
    in_maps = []
    for e in range(n_cores):
        in_maps.append({
            "h": np.ascontiguousarray(hidden_states, dtype=np.float32),
            "gate_w": np.ascontiguousarray(gate_w, dtype=np.float32),
            "w1l": np.ascontiguousarray(w1[e], dtype=np.float32),
            "w3l": np.ascontiguousarray(w3[e], dtype=np.float32),
            "w2l": np.ascontiguousarray(w2[e], dtype=np.float32),
            "onehot": np.tile(onehots[e], (128, 1)),
        })
    res = run_bass_kernel_spmd(nc, in_maps, list(range(n_cores))).results
    return np.concatenate([res[i]["out_shard"] for i in range(n_cores)], axis=0)


if __name__ == "__main__":
    nc = build_kernel()
    print("built", len(nc.inst_map), "instructions")


# revision 25
# speedup vs baseline: 1.4565x; 1.0493x over previous
"""Mixtral MoE layer (top-2 of 8 experts) on 8 Trainium2 NeuronCores.

Strategy: expert parallelism. Core e owns expert e's weights (w1/w3/w2[e]).
Each core:
  1. Router (exact fp32): logits = h @ gate_w, top-2 via max8, combine weight
     for own expert via sigmoid(l_e - l_other); builds a compaction rank for
     the tokens routed to this expert (matmul-based prefix sums).
  2. Compaction: payload rows [h | combine | token_id] are indirect-DMA
     scattered into a dense per-expert buffer h_c (capacity TCAP).
  3. FFN over compact tokens (fp32r stage A, bf16 stage B), scaled by the
     combine weight, indirect-scattered to the token's row of a [T,H] buffer.
  4. ReduceScatter(add) across the 8 cores; host concatenates the shards.
"""
import sys

sys.path.insert(0, "/opt/trn_rl_repo")

import numpy as np

import concourse.bass as bass
import concourse.mybir as mybir
from concourse import bacc
from concourse.tile import TileContext
from concourse.tile_rust import add_dep_helper
from concourse.masks import make_identity
from concourse.bass_utils import run_bass_kernel_spmd

F32 = mybir.dt.float32
F32R = mybir.dt.float32r
BF16 = mybir.dt.bfloat16
I32 = mybir.dt.int32
AF = mybir.ActivationFunctionType
P = 128


def build_kernel(T=16384, H=1024, FF=3584, E=8, TCAP=4608, CH=512, n_cores=8):
    NT = T // P      # token tiles
    KH = H // P      # contraction tiles over H
    KF = FF // P     # f tiles (stage A output tiles / stage B contraction)
    NCH = TCAP // CH
    CT = CH // P     # token tiles per FFN chunk
    WPAY = H + 8     # payload row: h | combine | token_id | pad
    TRASH = float(T)  # scatter row for capacity-pad slots
    BIG = 1.0e9
    NHALF = max(1, H // 512)  # stage B free-dim chunks
    HW2 = H // NHALF

    nc = bacc.Bacc(num_devices=n_cores, num_swdge_queues=4)

    h_ext = nc.dram_tensor("h", [T, H], F32, kind="ExternalInput")
    gw_ext = nc.dram_tensor("gate_w", [H, E], F32, kind="ExternalInput")
    w1_ext = nc.dram_tensor("w1l", [H, FF], F32R, kind="ExternalInput")
    w3_ext = nc.dram_tensor("w3l", [H, FF], F32R, kind="ExternalInput")
    w2_ext = nc.dram_tensor("w2l", [FF, H], F32, kind="ExternalInput")
    oh_ext = nc.dram_tensor("onehot", [P, E], F32, kind="ExternalInput")
    out_ext = nc.dram_tensor("out_shard", [T // n_cores, H], F32, kind="ExternalOutput")

    h_c = nc.dram_tensor("h_c", [TCAP, WPAY], F32)
    scat = nc.dram_tensor("scat", [T + P, H], BF16)
    rs_out = nc.dram_tensor("rs_out", [T // n_cores, H], BF16)

    tok_ids = np.arange(T, dtype=np.float32).reshape(NT, P).T.copy()  # [P, NT]
    tok_const = nc.inline_tensor(tok_ids, name="tok_ids")
    ustrict_np = np.triu(np.ones((P, P), dtype=np.float32), 1)  # [k, m] = 1 iff k < m
    ustrict_const = nc.inline_tensor(ustrict_np, name="ustrict")

    with TileContext(nc) as tc:
        with tc.tile_pool(name="const", bufs=1) as cpool:
            ident = cpool.tile([P, P], F32)
            make_identity(nc, ident[:])
            ustrict = cpool.tile([P, P], F32)
            nc.sync.dma_start(out=ustrict[:], in_=ustrict_const[:])
            tok_slab = cpool.tile([P, NT], F32)
            nc.sync.dma_start(out=tok_slab[:], in_=tok_const[:])
            ones_col = cpool.tile([P, 1], F32)
            nc.vector.memset(ones_col[:], 1.0)
            ones_row = cpool.tile([1, P], F32)
            nc.vector.memset(ones_row[:], 1.0)
            gw_sb = cpool.tile([P, KH, E], F32)
            nc.sync.dma_start(out=gw_sb[:], in_=gw_ext[:].rearrange("(k p) e -> p k e", p=P))
            oh_sb = cpool.tile([P, E], F32)
            nc.sync.dma_start(out=oh_sb[:], in_=oh_ext[:])
            zrow = cpool.tile([P, WPAY], F32)
            nc.vector.memset(zrow[:], 0.0)
            nc.vector.memset(zrow[:, H + 1:H + 2], TRASH)
            zrow_b = cpool.tile([P, H], BF16)
            nc.vector.memset(zrow_b[:], 0.0)
            zer_row = cpool.tile([1, P], F32)
            nc.vector.memset(zer_row[:], 0.0)

            # -------- router + compaction + payload, in overlapped groups --------
            # Tokens are processed in NG groups of GT tiles. Each group computes
            # its logits/top2/combine, then its compaction ranks; the global rank
            # base is carried between groups by chaining tensor_tensor_scan
            # (initial = previous group's last inclusive prefix). A group's
            # payload scatter only depends on its own ranks, so it overlaps the
            # next group's router compute/DMA instead of serializing at the end.
            # -------- zero-fill h_c and scat (batched, SWDGE queues) --------
            ZB = 4
            for r in range(TCAP // (P * ZB)):
                nc.gpsimd.dma_start(
                    out=h_c[r * P * ZB:(r + 1) * P * ZB, :].rearrange(
                        "(a p) w -> p a w", p=P),
                    in_=zrow[:, None, :].to_broadcast([P, ZB, WPAY]))
            NSC = (T + P) // P
            for r in range(NSC // ZB):
                nc.gpsimd.dma_start(
                    out=scat[r * P * ZB:(r + 1) * P * ZB, :].rearrange(
                        "(a p) w -> p a w", p=P),
                    in_=zrow_b[:, None, :].to_broadcast([P, ZB, H]))
            for r in range((NSC // ZB) * ZB, NSC):
                nc.gpsimd.dma_start(out=scat[r * P:(r + 1) * P, :], in_=zrow_b[:])

            GT = min(16, NT)
            NG = NT // GT
            scatter_insts = []
            with tc.tile_pool(name="rslab", bufs=1) as spool:
                mx_slab = spool.tile([P, NT, 8], F32)
                comb_slab = spool.tile([P, NT], F32)
                rank_i = spool.tile([P, NT], I32)
                cs_slab = spool.tile([1, NT], F32)
                incl_slab = spool.tile([1, NT], F32)

                with tc.tile_pool(name="rtile", bufs=3) as rpool, \
                     tc.tile_pool(name="rgrp", bufs=2) as gpool_r, \
                     tc.tile_pool(name="ppool", bufs=8) as ppool, \
                     tc.tile_pool(name="rpsum", bufs=2, space="PSUM") as rpsum, \
                     tc.tile_pool(name="rcpsum", bufs=1, space="PSUM") as rcpsum, \
                     tc.tile_pool(name="lgpsum", bufs=1, space="PSUM") as lgpsum:
                    SG = 4  # token tiles per logits subgroup (512-token strips)
                    for q in range(NG):
                        i0 = q * GT
                        lg_g = gpool_r.tile([P, GT, E], F32, tag="lg_g")
                        for s4 in range(GT // SG):
                            hT4 = rpool.tile([P, KH, SG * P], F32, tag="hT4")
                            for j4 in range(SG):
                                i = i0 + s4 * SG + j4
                                ht = rpool.tile([P, H], F32, tag="ht")
                                heng = nc.sync if i % 2 == 0 else nc.scalar
                                heng.dma_start(out=ht[:], in_=h_ext[i * P:(i + 1) * P, :])
                                trp = rpsum.tile([P, KH, P], F32, tag="trp")
                                for k in range(KH):
                                    nc.tensor.transpose(out=trp[:, k],
                                                        in_=ht[:, k * P:(k + 1) * P],
                                                        identity=ident[:])
                                dst = hT4[:, :, j4 * P:(j4 + 1) * P]
                                if i % 2 == 0:
                                    nc.vector.tensor_copy(out=dst, in_=trp[:])
                                else:
                                    nc.scalar.copy(out=dst, in_=trp[:])
                            # logits for 512 tokens with gate_w stationary: [8, 512]
                            lgT = lgpsum.tile([E, SG * P], F32, tag="lgT")
                            for k in range(KH):
                                nc.tensor.matmul(lgT[:], lhsT=gw_sb[:, k], rhs=hT4[:, k],
                                                 start=(k == 0), stop=(k == KH - 1))
                            lgT_sb = gpool_r.tile([E, SG * P], F32, tag="lgT_sb")
                            nc.vector.tensor_copy(out=lgT_sb[:], in_=lgT[:])
                            for t4 in range(SG):
                                i = i0 + s4 * SG + t4
                                lg = rcpsum.tile([P, E], F32, tag="lg")
                                nc.tensor.transpose(out=lg[:],
                                                    in_=lgT_sb[:, t4 * P:(t4 + 1) * P],
                                                    identity=ident[0:E, 0:E])
                                j = s4 * SG + t4
                                nc.scalar.copy(out=lg_g[:, j], in_=lg[:])
                                nc.vector.max(out=mx_slab[:, i], in_=lg_g[:, j])

                        # group combine/mask
                        sl = slice(i0, i0 + GT)
                        tmp_le = gpool_r.tile([P, GT, E], F32, tag="tmp_le")
                        nc.vector.tensor_mul(out=tmp_le[:], in0=lg_g[:],
                                             in1=oh_sb[:, None, :].to_broadcast([P, GT, E]))
                        le = gpool_r.tile([P, GT], F32, tag="le")
                        nc.vector.tensor_reduce(out=le[:], in_=tmp_le[:],
                                                axis=mybir.AxisListType.X,
                                                op=mybir.AluOpType.add)
                        m1 = mx_slab[:, sl, 0]
                        m2 = mx_slab[:, sl, 1]
                        msum = gpool_r.tile([P, GT], F32, tag="msum")
                        nc.vector.tensor_add(out=msum[:], in0=m1, in1=m2)
                        sgin = gpool_r.tile([P, GT], F32, tag="sgin")
                        nc.vector.tensor_scalar_mul(sgin[:], le[:], 2.0)
                        nc.vector.tensor_sub(out=sgin[:], in0=sgin[:], in1=msum[:])
                        sig = gpool_r.tile([P, GT], F32, tag="sig")
                        nc.scalar.activation(sig[:], sgin[:], AF.Sigmoid)
                        eq1 = gpool_r.tile([P, GT], F32, tag="eq1")
                        eq2 = gpool_r.tile([P, GT], F32, tag="eq2")
                        nc.vector.tensor_tensor(out=eq1[:], in0=le[:], in1=m1,
                                                op=mybir.AluOpType.is_equal)
                        nc.vector.tensor_tensor(out=eq2[:], in0=le[:], in1=m2,
                                                op=mybir.AluOpType.is_equal)
                        mask_g = gpool_r.tile([P, GT], F32, tag="mask_g")
                        nc.vector.tensor_add(out=mask_g[:], in0=eq1[:], in1=eq2[:])
                        nc.vector.tensor_mul(out=comb_slab[:, sl], in0=mask_g[:], in1=sig[:])

                        # group compaction ranks with chained global base
                        csum_ps = rcpsum.tile([1, GT], F32, tag="c1")
                        nc.tensor.matmul(csum_ps[:], lhsT=ones_col[:], rhs=mask_g[:],
                                         start=True, stop=True)
                        nc.vector.tensor_copy(out=cs_slab[:, sl], in_=csum_ps[:])
                        init = 0.0 if q == 0 else incl_slab[:, i0 - 1:i0]
                        nc.vector.tensor_tensor_scan(out=incl_slab[:, sl],
                                                     data0=cs_slab[:, sl],
                                                     data1=zer_row[:, 0:GT],
                                                     initial=init,
                                                     op0=mybir.AluOpType.add,
                                                     op1=mybir.AluOpType.add)
                        cpref = gpool_r.tile([1, GT], F32, tag="cpref")
                        nc.vector.tensor_sub(out=cpref[:], in0=incl_slab[:, sl],
                                             in1=cs_slab[:, sl])
                        rank_ps = rcpsum.tile([P, GT], F32, tag="rk")
                        nc.tensor.matmul(rank_ps[:], lhsT=ustrict[:], rhs=mask_g[:],
                                         start=True, stop=False)
                        nc.tensor.matmul(rank_ps[:], lhsT=ones_row[:], rhs=cpref[:],
                                         start=False, stop=True)
                        pad_off = gpool_r.tile([P, GT], F32, tag="pad_off")
                        nc.vector.tensor_scalar(out=pad_off[:], in0=mask_g[:],
                                                scalar1=-BIG, scalar2=BIG,
                                                op0=mybir.AluOpType.mult,
                                                op1=mybir.AluOpType.add)
                        rank_f = gpool_r.tile([P, GT], F32, tag="rank_f")
                        nc.vector.tensor_add(out=rank_f[:], in0=rank_ps[:], in1=pad_off[:])
                        nc.vector.tensor_copy(out=rank_i[:, sl], in_=rank_f[:])

                        # group payload scatter (overlaps next group's router)
                        for j in range(GT):
                            i = i0 + j
                            pay = ppool.tile([P, WPAY], F32, tag="pay")
                            eng = nc.scalar if i % 2 == 0 else nc.sync
                            eng.dma_start(out=pay[:, 0:H], in_=h_ext[i * P:(i + 1) * P, :])
                            nc.vector.tensor_copy(out=pay[:, H:H + 1],
                                                  in_=comb_slab[:, i:i + 1])
                            nc.vector.tensor_copy(out=pay[:, H + 1:H + 2],
                                                  in_=tok_slab[:, i:i + 1])
                            claim = bass.AP(
                                tensor=h_c[0:P, :].tensor, offset=0,
                                ap=h_c[0:P, :].ap,
                                dep_tracking_offset=(i % (TCAP // P)) * P * WPAY)
                            sc = nc.gpsimd.indirect_dma_start(
                                out=claim,
                                out_offset=bass.IndirectOffsetOnAxis(
                                    ap=rank_i[:, i:i + 1], axis=0),
                                in_=pay[:], in_offset=None,
                                bounds_check=TCAP - 1, oob_is_err=False)
                            sc.ins.queue = "qPoolDynamic" + str(i % 4 or '')
                            scatter_insts.append(sc.ins)

            # fence: all payload scatters complete before any h_c chunk read
            fence = nc.gpsimd.nop(hint="hc_fence", nofuse=True)
            for si in scatter_insts:
                add_dep_helper(fence.ins, si, True, "hc scatter fence")

            # -------- FFN over compact tokens --------
            with tc.tile_pool(name="fpool", bufs=2) as fpool, \
                 tc.tile_pool(name="wpool", bufs=4) as wpool, \
                 tc.tile_pool(name="hcpool", bufs=CT + 2) as hcpool, \
                 tc.tile_pool(name="gpool", bufs=KF) as gpool, \
                 tc.tile_pool(name="w2pool", bufs=KF) as w2pool, \
                 tc.tile_pool(name="opool", bufs=3) as opool, \
                 tc.tile_pool(name="ftrpsum", bufs=1, space="PSUM") as ftrpsum, \
                 tc.tile_pool(name="fpsum", bufs=2, space="PSUM") as fpsum, \
                 tc.tile_pool(name="opsum", bufs=1, space="PSUM") as opsum:

                # w2 resident in bf16
                w2b = []
                for f in range(KF):
                    w2s = fpool.tile([P, H], F32, tag="w2stage")
                    nc.sync.dma_start(out=w2s[:], in_=w2_ext[f * P:(f + 1) * P, :])
                    w2t = w2pool.tile([P, H], BF16, tag="w2b")
                    nc.vector.tensor_copy(out=w2t[:], in_=w2s[:])
                    w2b.append(w2t)

                for c in range(NCH):
                    hcts = []
                    idxs = []
                    for t in range(CT):
                        hct = hcpool.tile([P, WPAY], F32, tag="hc")
                        r0 = c * CH + t * P
                        ld = nc.sync.dma_start(out=hct[:], in_=h_c[r0:r0 + P, :])
                        add_dep_helper(ld.ins, fence.ins, True, "hc fence")
                        idx = hcpool.tile([P, 1], I32, tag="idx")
                        nc.vector.tensor_copy(out=idx[:], in_=hct[:, H + 1:H + 2])
                        hcts.append(hct)
                        idxs.append(idx)
                    hTr = fpool.tile([P, KH, CH], F32R, tag="hTr")
                    for t in range(CT):
                        trp = ftrpsum.tile([P, KH, P], F32, tag="ftr")
                        for k in range(KH):
                            nc.tensor.transpose(out=trp[:, k], in_=hcts[t][:, k * P:(k + 1) * P],
                                                identity=ident[:])
                        nc.vector.tensor_copy(out=hTr[:, :, t * P:(t + 1) * P], in_=trp[:])

                    # stage A: G^T tiles [f, tokens]
                    gts = []
                    for f in range(KF):
                        w1s = wpool.tile([P, KH, P], F32R, tag="w1s")
                        nc.sync.dma_start(
                            out=w1s[:],
                            in_=w1_ext[:, f * P:(f + 1) * P].rearrange("(k p) m -> p k m", p=P))
                        w3s = wpool.tile([P, KH, P], F32R, tag="w3s")
                        nc.sync.dma_start(
                            out=w3s[:],
                            in_=w3_ext[:, f * P:(f + 1) * P].rearrange("(k p) m -> p k m", p=P))
                        x1 = fpsum.tile([P, CH], F32, tag="x1")
                        x3 = fpsum.tile([P, CH], F32, tag="x3")
                        for k in range(KH):
                            nc.tensor.matmul(x1[:], lhsT=w1s[:, k], rhs=hTr[:, k],
                                             start=(k == 0), stop=(k == KH - 1))
                        for k in range(KH):
                            nc.tensor.matmul(x3[:], lhsT=w3s[:, k], rhs=hTr[:, k],
                                             start=(k == 0), stop=(k == KH - 1))
                        gate = fpool.tile([P, CH], F32, tag="gate")
                        nc.scalar.activation(gate[:], x1[:], AF.Silu)
                        gt = gpool.tile([P, CH], BF16, tag="G")
                        nc.vector.tensor_mul(out=gt[:], in0=gate[:], in1=x3[:])
                        gts.append(gt)

                    # stage B: out rows, scaled by combine, scattered to scat
                    for t in range(CT):
                        o = opsum.tile([P, H], F32, tag="o")
                        for f in range(KF):
                            for hh in range(NHALF):
                                nc.tensor.matmul(
                                    o[:, hh * HW2:(hh + 1) * HW2],
                                    lhsT=gts[f][:, t * P:(t + 1) * P],
                                    rhs=w2b[f][:, hh * HW2:(hh + 1) * HW2],
                                    start=(f == 0), stop=(f == KF - 1))
                        osb = opool.tile([P, H], BF16, tag="osb")
                        nc.vector.tensor_scalar_mul(osb[:], o[:], hcts[t][:, H:H + 1])
                        oclaim = bass.AP(
                            tensor=scat[0:P, :].tensor, offset=0,
                            ap=scat[0:P, :].ap,
                            dep_tracking_offset=(c * CT + t) * P * H)
                        sco = nc.gpsimd.indirect_dma_start(
                            out=oclaim,
                            out_offset=bass.IndirectOffsetOnAxis(ap=idxs[t][:, 0:1], axis=0),
                            in_=osb[:], in_offset=None,
                            bounds_check=T + P - 1, oob_is_err=False)
                        sco.ins.queue = "qPoolDynamic" + str((c * CT + t) % 4 or '')

            # -------- collective + output --------
            with tc.tile_pool(name="oc", bufs=2) as ocpool:
                nc.gpsimd.collective_compute(
                    "ReduceScatter", mybir.AluOpType.add,
                    replica_groups=[list(range(n_cores))],
                    ins=[scat[0:T, :]], outs=[rs_out[:]])
                shard = T // n_cores
                for r in range(shard // P):
                    oct_ = ocpool.tile([P, H], BF16, tag="oct")
                    nc.sync.dma_start(out=oct_[:], in_=rs_out[r * P:(r + 1) * P, :])
                    octf = ocpool.tile([P, H], F32, tag="octf")
                    nc.vector.tensor_copy(out=octf[:], in_=oct_[:])
                    nc.sync.dma_start(out=out_ext[r * P:(r + 1) * P, :], in_=octf[:])

    nc.finalize()
    return nc


def kernel(hidden_states, gate_w, w1, w3, w2):
    T, H = hidden_states.shape
    E, _, FF = w1.shape
    n_cores = 8
    nc = build_kernel(T=T, H=H, FF=FF, E=E, n_cores=n_cores)
    onehots = np.eye(E, dtype=np.float32)
    in_maps = []
    for e in range(n_cores):
        in_maps.append({
            "h": np.ascontiguousarray(hidden_states, dtype=np.float32),
            "gate_w": np.ascontiguousarray(gate_w, dtype=np.float32),
            "w1l": np.ascontiguousarray(w1[e], dtype=np.float32),
            "w3l": np.ascontiguousarray(w3[e], dtype=np.float32),
            "w2l": np.ascontiguousarray(w2[e], dtype=np.float32),
            "onehot": np.tile(onehots[e], (128, 1)),
        })
    res = run_bass_kernel_spmd(nc, in_maps, list(range(n_cores))).results
    return np.concatenate([res[i]["out_shard"] for i in range(n_cores)], axis=0)


if __name__ == "__main__":
    nc = build_kernel()
    print("built", len(nc.inst_map), "instructions")
